# revision 1
# baseline (speedup 1.0000x reference)
import sys
sys.path.insert(0, '/opt/trn_rl_repo')
import numpy as np
import math

import concourse.bass as bass
import concourse.mybir as mybir
import concourse.tile as tile
from concourse import bacc
from concourse.bass_utils import run_bass_kernel_spmd

# Problem dims
B, SL, CH, HZ = 128, 5000, 12, 100
L, D, DFF, H, NCLS = 5, 1024, 4096, 16, 71
NI = CH * HZ          # 1200
S = SL // HZ          # 50
NCORES = 8
NB = B // NCORES      # 16 batches per core
T = NB * S            # 800 tokens per core
NIP = 1280            # padded input-feature dim (1200 + 50 one-hot -> 1250 -> 1280)
NKI = NIP // 128      # 10 input k-chunks
DK = D // H           # 64
NDC = D // 128        # 8 d-chunks
NFC = DFF // 128      # 32 ff-chunks
HB = NB // 2          # 8 batches per half
HT = HB * S           # 400 tokens per half

F32R = mybir.dt.float32r
F32 = mybir.dt.float32
BF16 = mybir.dt.bfloat16
EXP = mybir.ActivationFunctionType.Exp
RELU = mybir.ActivationFunctionType.Relu
AOP = mybir.AluOpType

TRACE = False
LAST_EXEC_NS = None
_CACHE = {}
POOL_CFG = dict(sq=2, wsm=6, wst=2, pt=2, ptn=2, rden=2, odd=1, psmm=3, psat=3, psrow=2)


def _build(n_layers=L):
    nc = bacc.Bacc(None)
    d = {}
    d['xT'] = nc.dram_tensor("xT", [NIP, T], F32R, kind="ExternalInput")
    d['ones'] = nc.dram_tensor("ones", [128, 512], F32R, kind="ExternalInput")
    d['emb_w'] = nc.dram_tensor("emb_w", [NDC, 128, NKI * 128], F32R, kind="ExternalInput")
    d['qkv_w'] = nc.dram_tensor("qkv_w", [L, 3, NDC, 128, NDC * 128], F32R, kind="ExternalInput")
    d['qkv_bT'] = nc.dram_tensor("qkv_bT", [L, 128, 3 * NDC], F32, kind="ExternalInput")
    d['wv_nat'] = nc.dram_tensor("wv_nat", [L, NDC, 128, D], F32R, kind="ExternalInput")
    d['wo_w'] = nc.dram_tensor("wo_w", [L, NDC, 128, NDC * 128], F32R, kind="ExternalInput")
    d['wo_b'] = nc.dram_tensor("wo_b", [L, NDC, 1, 128], F32R, kind="ExternalInput")
    d['w1_w'] = nc.dram_tensor("w1_w", [L, NFC, 128, NDC * 128], F32R, kind="ExternalInput")
    d['w1_bT'] = nc.dram_tensor("w1_bT", [L, 128, NFC], F32, kind="ExternalInput")
    d['w2_w'] = nc.dram_tensor("w2_w", [L, NDC, 128, NFC * 128], F32R, kind="ExternalInput")
    d['w2_b'] = nc.dram_tensor("w2_b", [L, NDC, 1, 128], F32R, kind="ExternalInput")
    d['cf_w'] = nc.dram_tensor("cf_w", [NDC, 128, NDC * 128], F32R, kind="ExternalInput")
    d['cf_bT'] = nc.dram_tensor("cf_bT", [128, NDC], F32, kind="ExternalInput")
    d['fc_w'] = nc.dram_tensor("fc_w", [128, NDC * NCLS], F32R, kind="ExternalInput")
    d['fc_b'] = nc.dram_tensor("fc_b", [NCLS, 1], F32, kind="ExternalInput")
    out = nc.dram_tensor("out", [NCLS, NB], F32, kind="ExternalOutput")

    with tile.TileContext(nc) as tc:
        _emit(nc, tc, d, out, n_layers)
    nc.compile()
    return nc


def _emit(nc, tc, d, out, n_layers):
    import contextlib
    ctx = contextlib.ExitStack()
    with ctx:
        sb1 = ctx.enter_context(tc.tile_pool(name="sb1", bufs=1))
        sq_p = ctx.enter_context(tc.tile_pool(name="sqp", bufs=POOL_CFG["sq"]))
        wsm = ctx.enter_context(tc.tile_pool(name="wsm", bufs=POOL_CFG["wsm"]))
        wst = ctx.enter_context(tc.tile_pool(name="wst", bufs=POOL_CFG["wst"]))
        rows = ctx.enter_context(tc.tile_pool(name="rows", bufs=4))
        rden_p = ctx.enter_context(tc.tile_pool(name="rden", bufs=POOL_CFG["rden"]))
        brow_p = ctx.enter_context(tc.tile_pool(name="brow", bufs=2))
        pt_p = ctx.enter_context(tc.tile_pool(name="ptp", bufs=POOL_CFG["pt"]))
        ptn_p = ctx.enter_context(tc.tile_pool(name="ptnp", bufs=POOL_CFG["ptn"]))
        odd_p = ctx.enter_context(tc.tile_pool(name="oddp", bufs=POOL_CFG["odd"]))
        ps_mm = ctx.enter_context(tc.tile_pool(name="psmm", bufs=POOL_CFG["psmm"], space="PSUM"))
        ps_at = ctx.enter_context(tc.tile_pool(name="psat", bufs=POOL_CFG["psat"], space="PSUM"))
        ps_row = ctx.enter_context(tc.tile_pool(name="psrow", bufs=POOL_CFG["psrow"], space="PSUM"))

        # persistent tiles
        hT = sb1.tile([128, NDC, T], F32R, tag="hT")
        ones_c = sb1.tile([128, 1], F32R, tag="ones_c")
        ones_r = sb1.tile([1, 512], F32R, tag="ones_r")
        nc.sync.dma_start(ones_c[:], d['ones'][:, 0:1])
        nc.sync.dma_start(ones_r[:], d['ones'][0:1, :])

        D_ = float(D)
        c_mean = 1.0 / D_
        c_v2 = 1.0 / (D_ - 1.0)
        c_v1 = -1.0 / (D_ * (D_ - 1.0))

        def ln_half(src, hcol0, ncols, dst, nch):
            """Plain LN over feature dim (nch*128) of src[:, :, hcol0:hcol0+ncols] -> dst[:, :, 0:ncols].
            src/dst are [128, nch, *] fp32r tiles."""
            Dn = float(nch * 128)
            cm = 1.0 / Dn
            cv2 = 1.0 / (Dn - 1.0)
            cv1 = -1.0 / (Dn * (Dn - 1.0))
            s1 = ps_row.tile([1, ncols], F32, tag="row")
            s2 = ps_row.tile([1, ncols], F32, tag="row")
            for c in range(nch):
                sq = sq_p.tile([128, ncols], F32R, tag="sq")
                nc.scalar.square(sq[:], src[:, c, hcol0:hcol0 + ncols])
                nc.tensor.matmul(s1[:], ones_c[:], src[:, c, hcol0:hcol0 + ncols],
                                 start=(c == 0), stop=(c == nch - 1))
                nc.tensor.matmul(s2[:], ones_c[:], sq[:],
                                 start=(c == 0), stop=(c == nch - 1))
            m_row = rows.tile([1, ncols], F32R, tag="rowsb")
            t1 = rows.tile([1, ncols], F32, tag="rowsb")
            t2 = rows.tile([1, ncols], F32, tag="rowsb")
            nc.vector.tensor_scalar_mul(m_row[:], s1[:], cm)
            nc.scalar.square(t1[:], s1[:])
            nc.vector.tensor_scalar_mul(t1[:], t1[:], cv1)
            nc.vector.tensor_scalar_mul(t2[:], s2[:], cv2)
            nc.vector.tensor_tensor(out=t1[:], in0=t1[:], in1=t2[:], op=AOP.add)
            nc.scalar.sqrt(t1[:], t1[:])
            nc.vector.tensor_scalar_add(t1[:], t1[:], 1e-6)
            r_row = rows.tile([1, ncols], F32R, tag="rowsb")
            with nc.allow_low_precision(reason="fp32r rounding of 1/(std+eps)"):
                nc.vector.reciprocal(r_row[:], t1[:])
            Mb = ps_at.tile([128, ncols], F32, tag="at")
            Rb = ps_at.tile([128, ncols], F32, tag="at")
            nc.tensor.matmul(Mb[:], ones_r[0:1, 0:128], m_row[:], start=True, stop=True)
            nc.tensor.matmul(Rb[:], ones_r[0:1, 0:128], r_row[:], start=True, stop=True)
            for c in range(nch):
                nc.vector.tensor_tensor(out=dst[:, c, 0:ncols], in0=src[:, c, hcol0:hcol0 + ncols],
                                        in1=Mb[:], op=AOP.subtract)
                nc.vector.tensor_tensor(out=dst[:, c, 0:ncols], in0=dst[:, c, 0:ncols],
                                        in1=Rb[:], op=AOP.mult)

        # ---------------- embed ----------------
        xt = sb1.tile([128, NKI, T], F32R, tag="tagV")
        nc.sync.dma_start(
            xt[:],
            d['xT'].rearrange("(k p) t -> p k t", p=128))
        for m in range(NDC):
            wt = wst.tile([128, NKI, 128], F32R, tag="wst")
            nc.sync.dma_start(wt[:], d['emb_w'][m].rearrange("p (k c) -> p k c", k=NKI))
            for hf in range(2):
                ps = ps_mm.tile([128, HT], F32, tag="mm")
                for k in range(NKI):
                    nc.tensor.matmul(ps[:], wt[:, k, :], xt[:, k, hf * HT:(hf + 1) * HT],
                                     start=(k == 0), stop=(k == NKI - 1))
                nc.vector.tensor_copy(hT[:, m, hf * HT:(hf + 1) * HT], ps[:])

        # ---------------- layers ----------------
        for li in range(n_layers):
            last = (li == n_layers - 1) and (n_layers == L)
            # ---- attention (per token-half) ----
            for hf in range(2):
                hc0 = hf * HT
                aT = sb1.tile([128, NDC, HT], F32R, tag="tagA")
                ln_half(hT, hc0, HT, aT, NDC)
                # Q, K
                qT = sb1.tile([128, NDC, HT], F32R, tag="tagQ")
                kT = sb1.tile([128, NDC, HT], F32R, tag="tagK")
                bT = brow_p.tile([128, 2 * NDC], F32, tag="brow")
                nc.sync.dma_start(bT[:], d['qkv_bT'][li, :, 0:2 * NDC])
                for mat, dst in ((0, qT), (1, kT)):
                    for m in range(NDC):
                        wt = wsm.tile([128, NDC, 128], F32R, tag="wsm")
                        nc.sync.dma_start(wt[:], d['qkv_w'][li, mat, m].rearrange("p (k c) -> p k c", k=NDC))
                        ps = ps_mm.tile([128, HT], F32, tag="mm")
                        for k in range(NDC):
                            nc.tensor.matmul(ps[:], wt[:, k, :], aT[:, k, :],
                                             start=(k == 0), stop=(k == NDC - 1))
                        nc.vector.tensor_scalar_add(
                            dst[:, m, :], ps[:],
                            bT[:, mat * NDC + m:mat * NDC + m + 1])
                # V (no bias; folded into wo_b on host): v[b] token-major [50, 1024]
                v = sb1.tile([64, HB, D], F32R, tag="tagV")
                wvA = wst.tile([128, 4, D], F32R, tag="wst")
                wvB = wst.tile([128, 4, D], F32R, tag="wst")
                nc.sync.dma_start(wvA[:], d['wv_nat'][li, 0:4].rearrange("k p n -> p k n"))
                nc.sync.dma_start(wvB[:], d['wv_nat'][li, 4:8].rearrange("k p n -> p k n"))
                for bi in range(HB):
                    bc0 = bi * S
                    for n in range(2):
                        ps = ps_mm.tile([128, 512], F32, tag="mm")
                        for k in range(NDC):
                            wv = wvA if k < 4 else wvB
                            nc.tensor.matmul(ps[0:S, :], aT[:, k, bc0:bc0 + S],
                                             wv[:, k % 4, n * 512:(n + 1) * 512],
                                             start=(k == 0), stop=(k == NDC - 1))
                        nc.vector.tensor_copy(v[0:S, bi, n * 512:(n + 1) * 512], ps[0:S, :])
                # attention per batch
                oT = sb1.tile([128, NDC, HT], F32R, tag="tagO")
                for bi in range(HB):
                    bc0 = bi * S
                    psE = ps_at.tile([S, 8 * S], F32, tag="at")
                    psO = ps_at.tile([S, 8 * S], F32, tag="at")
                    for c in range(NDC):
                        nc.tensor.matmul(psE[:, c * S:(c + 1) * S],
                                         kT[0:DK, c, bc0:bc0 + S], qT[0:DK, c, bc0:bc0 + S],
                                         start=True, stop=True)
                    for c in range(NDC):
                        nc.tensor.matmul(psO[:, c * S:(c + 1) * S],
                                         kT[DK:128, c, bc0:bc0 + S], qT[DK:128, c, bc0:bc0 + S],
                                         start=True, stop=True)
                    pTE = pt_p.tile([S, 8 * S], F32R, tag="pt")
                    pTO = pt_p.tile([S, 8 * S], F32R, tag="pt")
                    nc.scalar.activation(pTE[:], psE[:], EXP, bias=0.0, scale=1.0 / math.sqrt(DK))
                    nc.scalar.activation(pTO[:], psO[:], EXP, bias=0.0, scale=1.0 / math.sqrt(DK))
                    denE = ps_row.tile([1, 8 * S], F32, tag="row")
                    denO = ps_row.tile([1, 8 * S], F32, tag="row")
                    nc.tensor.matmul(denE[:], ones_c[0:S, :], pTE[:], start=True, stop=True)
                    nc.tensor.matmul(denO[:], ones_c[0:S, :], pTO[:], start=True, stop=True)
                    rdE = rden_p.tile([1, 8 * S], F32R, tag="rden")
                    rdO = rden_p.tile([1, 8 * S], F32R, tag="rden")
                    with nc.allow_low_precision(reason="softmax denom reciprocal"):
                        nc.vector.reciprocal(rdE[:], denE[:])
                        nc.vector.reciprocal(rdO[:], denO[:])
                    bcE = ps_at.tile([S, 8 * S], F32, tag="at")
                    bcO = ps_at.tile([S, 8 * S], F32, tag="at")
                    nc.tensor.matmul(bcE[:], ones_r[0:1, 0:S], rdE[:], start=True, stop=True)
                    nc.tensor.matmul(bcO[:], ones_r[0:1, 0:S], rdO[:], start=True, stop=True)
                    pnE = ptn_p.tile([S, 8 * S], F32R, tag="ptn")
                    pnO = ptn_p.tile([S, 8 * S], F32R, tag="ptn")
                    nc.vector.tensor_tensor(out=pnE[:], in0=pTE[:], in1=bcE[:], op=AOP.mult)
                    nc.vector.tensor_tensor(out=pnO[:], in0=pTO[:], in1=bcO[:], op=AOP.mult)
                    poE = ps_at.tile([DK, 8 * S], F32, tag="at")
                    poO = ps_at.tile([DK, 8 * S], F32, tag="at")
                    for c in range(NDC):
                        nc.tensor.matmul(poE[:, c * S:(c + 1) * S],
                                         v[0:S, bi, (2 * c) * DK:(2 * c + 1) * DK],
                                         pnE[:, c * S:(c + 1) * S], start=True, stop=True)
                    for c in range(NDC):
                        nc.tensor.matmul(poO[:, c * S:(c + 1) * S],
                                         v[0:S, bi, (2 * c + 1) * DK:(2 * c + 2) * DK],
                                         pnO[:, c * S:(c + 1) * S], start=True, stop=True)
                    # even heads -> oT rows 0-63 directly
                    nc.vector.tensor_copy(
                        oT[0:DK, :, bc0:bc0 + S],
                        poE[:].rearrange("p (c t) -> p c t", c=NDC))
                    # odd heads -> scratch -> DMA shift to rows 64-127
                    osc = odd_p.tile([DK, 8 * S], F32R, tag="odd")
                    nc.vector.tensor_copy(osc[:], poO[:])
                    nc.sync.dma_start(
                        oT[DK:128, :, bc0:bc0 + S],
                        osc[:].rearrange("p (c t) -> p c t", c=NDC))
                # Wo + residual
                for m in range(NDC):
                    wt = wsm.tile([128, NDC, 128], F32R, tag="wsm")
                    nc.sync.dma_start(wt[:], d['wo_w'][li, m].rearrange("p (k c) -> p k c", k=NDC))
                    br = brow_p.tile([1, 128], F32R, tag="brow2")
                    nc.sync.dma_start(br[:], d['wo_b'][li, m])
                    ps = ps_mm.tile([128, HT], F32, tag="mm")
                    nc.tensor.matmul(ps[:], br[:], ones_r[0:1, 0:HT], start=True, stop=False)
                    for k in range(NDC):
                        nc.tensor.matmul(ps[:], wt[:, k, :], oT[:, k, :],
                                         start=False, stop=(k == NDC - 1))
                    nc.vector.tensor_tensor(out=hT[:, m, hc0:hc0 + HT], in0=hT[:, m, hc0:hc0 + HT],
                                            in1=ps[:], op=AOP.add)
            # ---- FFN ----
            if not last:
                for hf in range(2):
                    hc0 = hf * HT
                    aT = sb1.tile([128, NDC, HT], F32R, tag="tagA")
                    ln_half(hT, hc0, HT, aT, NDC)
                    b1T = brow_p.tile([128, NFC], F32, tag="brow")
                    nc.sync.dma_start(b1T[:], d['w1_bT'][li])
                    ffq0 = sb1.tile([128, 8, HT], F32R, tag="tagQ")
                    ffq1 = sb1.tile([128, 8, HT], F32R, tag="tagK")
                    ffq2 = sb1.tile([128, 8, HT], F32R, tag="tagO")
                    ffq3 = sb1.tile([128, 8, HT], F32R, tag="tagF")
                    ffq = [ffq0, ffq1, ffq2, ffq3]
                    for m in range(NFC):
                        wt = wsm.tile([128, NDC, 128], F32R, tag="wsm")
                        nc.sync.dma_start(wt[:], d['w1_w'][li, m].rearrange("p (k c) -> p k c", k=NDC))
                        ps = ps_mm.tile([128, HT], F32, tag="mm")
                        for k in range(NDC):
                            nc.tensor.matmul(ps[:], wt[:, k, :], aT[:, k, :],
                                             start=(k == 0), stop=(k == NDC - 1))
                        nc.scalar.activation(ffq[m // 8][:, m % 8, :], ps[:], RELU,
                                             bias=b1T[:, m:m + 1], scale=1.0)
                    for m in range(NDC):
                        w2t = wst.tile([128, NFC, 128], F32R, tag="wst")
                        nc.sync.dma_start(w2t[:], d['w2_w'][li, m].rearrange("p (k c) -> p k c", k=NFC))
                        br = brow_p.tile([1, 128], F32R, tag="brow2")
                        nc.sync.dma_start(br[:], d['w2_b'][li, m])
                        ps = ps_mm.tile([128, HT], F32, tag="mm")
                        nc.tensor.matmul(ps[:], br[:], ones_r[0:1, 0:HT], start=True, stop=False)
                        for k in range(NFC):
                            nc.tensor.matmul(ps[:], w2t[:, k, :], ffq[k // 8][:, k % 8, :],
                                             start=False, stop=(k == NFC - 1))
                        nc.vector.tensor_tensor(out=hT[:, m, hc0:hc0 + HT],
                                                in0=hT[:, m, hc0:hc0 + HT],
                                                in1=ps[:], op=AOP.add)
            else:
                # last layer: FFN only for the last token of each batch
                hL = sb1.tile([128, NDC, NB], F32R, tag="hL")
                for c in range(NDC):
                    nc.vector.tensor_copy(
                        hL[:, c, :],
                        hT[:, c, :].rearrange("p (b s) -> p b s", s=S)[:, :, S - 1])
                aL = sb1.tile([128, NDC, NB], F32R, tag="aL")
                ln_half(hL, 0, NB, aL, NDC)
                b1T = brow_p.tile([128, NFC], F32, tag="brow")
                nc.sync.dma_start(b1T[:], d['w1_bT'][li])
                ffL = sb1.tile([128, NFC, NB], F32R, tag="ffL")
                for m in range(NFC):
                    wt = wsm.tile([128, NDC, 128], F32R, tag="wsm")
                    nc.sync.dma_start(wt[:], d['w1_w'][li, m].rearrange("p (k c) -> p k c", k=NDC))
                    ps = ps_mm.tile([128, NB], F32, tag="mm")
                    for k in range(NDC):
                        nc.tensor.matmul(ps[:], wt[:, k, :], aL[:, k, :],
                                         start=(k == 0), stop=(k == NDC - 1))
                    nc.scalar.activation(ffL[:, m, :], ps[:], RELU,
                                         bias=b1T[:, m:m + 1], scale=1.0)
                for m in range(NDC):
                    w2t = wst.tile([128, NFC, 128], F32R, tag="wst")
                    nc.sync.dma_start(w2t[:], d['w2_w'][li, m].rearrange("p (k c) -> p k c", k=NFC))
                    br = brow_p.tile([1, 128], F32R, tag="brow2")
                    nc.sync.dma_start(br[:], d['w2_b'][li, m])
                    ps = ps_mm.tile([128, NB], F32, tag="mm")
                    nc.tensor.matmul(ps[:], br[:], ones_r[0:1, 0:NB], start=True, stop=False)
                    for k in range(NFC):
                        nc.tensor.matmul(ps[:], w2t[:, k, :], ffL[:, k, :],
                                         start=False, stop=(k == NFC - 1))
                    nc.vector.tensor_tensor(out=hL[:, m, :], in0=hL[:, m, :],
                                            in1=ps[:], op=AOP.add)

        # ---------------- head ----------------
        if n_layers == L:
            src_pool = hL
        else:
            # debug path (fewer layers): extract last tokens now
            src_pool = sb1.tile([128, NDC, NB], F32R, tag="hL")
            for c in range(NDC):
                nc.vector.tensor_copy(
                    src_pool[:, c, :],
                    hT[:, c, :].rearrange("p (b s) -> p b s", s=S)[:, :, S - 1])
        pL = sb1.tile([128, NDC, NB], F32R, tag="pL")
        ln_half(src_pool, 0, NB, pL, NDC)
        cbT = brow_p.tile([128, NDC], F32, tag="brow")
        nc.sync.dma_start(cbT[:], d['cf_bT'][:])
        z1 = sb1.tile([128, NDC, NB], F32R, tag="z1")
        for m in range(NDC):
            wt = wsm.tile([128, NDC, 128], F32R, tag="wsm")
            nc.sync.dma_start(wt[:], d['cf_w'][m].rearrange("p (k c) -> p k c", k=NDC))
            ps = ps_mm.tile([128, NB], F32, tag="mm")
            for k in range(NDC):
                nc.tensor.matmul(ps[:], wt[:, k, :], pL[:, k, :],
                                 start=(k == 0), stop=(k == NDC - 1))
            nc.scalar.activation(z1[:, m, :], ps[:], RELU, bias=cbT[:, m:m + 1], scale=1.0)
        fwt = sb1.tile([128, NDC, NCLS], F32R, tag="fwt")
        nc.sync.dma_start(fwt[:], d['fc_w'].rearrange("p (k c) -> p k c", k=NDC))
        fb = brow_p.tile([NCLS, 1], F32, tag="brow2")
        nc.sync.dma_start(fb[:], d['fc_b'][:])
        ps = ps_mm.tile([NCLS, NB], F32, tag="mm")
        for k in range(NDC):
            nc.tensor.matmul(ps[:], fwt[:, k, :], z1[:, k, :],
                             start=(k == 0), stop=(k == NDC - 1))
        osb = sb1.tile([NCLS, NB], F32, tag="osb")
        nc.vector.tensor_scalar_add(osb[:], ps[:], fb[:])
        nc.sync.dma_start(out[:], osb[:])


def _prep_weights(inputs, n_layers=L):
    f64 = np.float64

    def prep_lhsT(W):
        # W [K, M] -> [M/128, 128, (K/128)*128] : tile[p, k*128+c] = W[k*128+p, mb*128+c]
        K, M = W.shape
        nk, nm = K // 128, M // 128
        return np.ascontiguousarray(
            W.reshape(nk, 128, nm, 128).transpose(2, 1, 0, 3).reshape(nm, 128, nk * 128)
        ).astype(np.float32)

    emb = inputs['embed_w'].astype(f64)          # [1200, 1024]
    # positional encoding rows appended via one-hot block
    pos = np.arange(S, dtype=f64)[:, None]
    div = np.exp(np.arange(0, D, 2, dtype=np.float32).astype(f64) * (-math.log(10000.0) / D))
    pe = np.zeros((S, D), f64)
    pe[:, 0::2] = np.sin(pos * div)
    pe[:, 1::2] = np.cos(pos * div)
    Wp = np.zeros((NIP, D), f64)
    Wp[:NI] = emb
    Wp[NI:NI + S] = pe
    g = {}
    g['emb_w'] = prep_lhsT(Wp)

    ln_g = inputs['ln_g'].astype(f64); ln_b = inputs['ln_b'].astype(f64)
    aw = inputs['attn_w'].astype(f64); ab = inputs['attn_b'].astype(f64)
    fw1 = inputs['ff_w1'].astype(f64); fb1 = inputs['ff_b1'].astype(f64)
    fw2 = inputs['ff_w2'].astype(f64); fb2 = inputs['ff_b2'].astype(f64)

    qkv_w = np.zeros((L, 3, NDC, 128, NDC * 128), np.float32)
    qkv_bT = np.zeros((L, 128, 3 * NDC), np.float32)
    wv_nat = np.zeros((L, NDC, 128, D), np.float32)
    wo_w = np.zeros((L, NDC, 128, NDC * 128), np.float32)
    wo_b = np.zeros((L, NDC, 1, 128), np.float32)
    w1_w = np.zeros((L, NFC, 128, NDC * 128), np.float32)
    w1_bT = np.zeros((L, 128, NFC), np.float32)
    w2_w = np.zeros((L, NDC, 128, NFC * 128), np.float32)
    w2_b = np.zeros((L, NDC, 1, 128), np.float32)

    for i in range(n_layers):
        g1, b1 = ln_g[i, 0][:, None], ln_b[i, 0]
        for mat in range(3):
            We = g1 * aw[i, mat]
            be = ab[i, mat] + b1 @ aw[i, mat]
            if mat == 2:
                wv_nat[i] = We.astype(np.float32).reshape(NDC, 128, D)
                # v bias folded into wo_b below (softmax rows sum to 1)
                bv = be
            else:
                qkv_w[i, mat] = prep_lhsT(We)
                qkv_bT[i, :, mat * NDC:(mat + 1) * NDC] = be.reshape(NDC, 128).T
        wo_w[i] = prep_lhsT(aw[i, 3])
        wo_be = ab[i, 3] + bv @ aw[i, 3]
        wo_b[i] = wo_be.reshape(NDC, 1, 128).transpose(0, 1, 2)
        g2, b2 = ln_g[i, 1][:, None], ln_b[i, 1]
        W1e = g2 * fw1[i]
        b1e = fb1[i] + b2 @ fw1[i]
        w1_w[i] = prep_lhsT(W1e)
        w1_bT[i] = b1e.reshape(NFC, 128).T
        w2_w[i] = prep_lhsT(fw2[i])
        w2_b[i] = fb2[i].reshape(NDC, 1, 128)

    g['qkv_w'] = qkv_w; g['qkv_bT'] = qkv_bT; g['wv_nat'] = wv_nat
    g['wo_w'] = wo_w; g['wo_b'] = wo_b
    g['w1_w'] = w1_w; g['w1_bT'] = w1_bT; g['w2_w'] = w2_w; g['w2_b'] = w2_b

    inv = 1.0 / math.sqrt(1.0 + 1e-5)
    fin_g = inputs['fin_g'].astype(f64); fin_b = inputs['fin_b'].astype(f64)
    A1 = fin_g * inv * inputs['cf_bn_g'].astype(f64)
    C1 = fin_b * inv * inputs['cf_bn_g'].astype(f64) + inputs['cf_bn_b'].astype(f64)
    cfw = inputs['cf_w'].astype(f64)
    cf_we = A1[:, None] * cfw
    cf_be = inputs['cf_b'].astype(f64) + C1 @ cfw
    g['cf_w'] = prep_lhsT(cf_we)
    g['cf_bT'] = cf_be.reshape(NDC, 128).T.astype(np.float32)
    A2 = inv * inputs['fc_bn_g'].astype(f64)
    C2 = inputs['fc_bn_b'].astype(f64)
    fcw = inputs['fc_w'].astype(f64)
    fc_we = A2[:, None] * fcw
    fc_be = inputs['fc_b'].astype(f64) + C2 @ fcw
    # fc_w [1024, 71] -> [128, 8*71]
    g['fc_w'] = np.ascontiguousarray(
        fc_we.reshape(NDC, 128, NCLS).transpose(1, 0, 2).reshape(128, NDC * NCLS)
    ).astype(np.float32)
    g['fc_b'] = fc_be.reshape(NCLS, 1).astype(np.float32)
    g['ones'] = np.ones((128, 512), np.float32)
    return g


def _run_timed(nc, in_maps, n_iters=10):
    """Mirror bass2jax.run_bass_via_pjrt (no donation), time steady-state execs."""
    import time
    import jax
    import numpy as _np
    from jax.experimental.shard_map import shard_map
    from jax.sharding import Mesh, PartitionSpec, NamedSharding
    from concourse import bass2jax as b2j
    from concourse import mybir as _mb

    b2j.install_neuronx_cc_hook()
    n_cores = len(in_maps)
    partition_name = nc.partition_id_tensor.name if nc.partition_id_tensor else None
    in_names, out_names, out_avals, zero_outs = [], [], [], []
    for alloc in nc.m.functions[0].allocations:
        if not isinstance(alloc, _mb.MemoryLocationSet):
            continue
        name = alloc.memorylocations[0].name
        if alloc.kind == "ExternalInput":
            if name != partition_name:
                in_names.append(name)
        elif alloc.kind == "ExternalOutput":
            shape = tuple(alloc.tensor_shape)
            dtype = _mb.dt.np(alloc.dtype)
            out_names.append(name)
            out_avals.append(jax.core.ShapedArray(shape, dtype))
            zero_outs.append(_np.zeros(shape, dtype))
    n_params = len(in_names)
    all_in_names = list(in_names) + list(out_names)
    if partition_name is not None:
        all_in_names.append(partition_name)

    def _body(*args):
        operands = list(args)
        if partition_name is not None:
            operands.append(b2j.partition_id_tensor())
        outs = b2j._bass_exec_p.bind(
            *operands,
            out_avals=tuple(out_avals),
            in_names=tuple(all_in_names),
            out_names=tuple(out_names),
            lowering_input_output_aliases=(),
            sim_require_finite=True,
            sim_require_nnan=True,
            nc=nc,
        )
        return tuple(outs)

    devices = jax.devices()[:n_cores]
    mesh = Mesh(_np.asarray(devices), ("core",))
    spec = PartitionSpec("core")
    sharded = jax.jit(shard_map(
        _body, mesh=mesh, in_specs=(spec,) * (n_params + len(out_names)),
        out_specs=(spec,) * len(out_names), check_rep=False))
    sh = NamedSharding(mesh, spec)
    concat_in = [
        jax.device_put(_np.concatenate([_np.asarray(m[name]) for m in in_maps], axis=0), sh)
        for name in in_names
    ]
    concat_zeros = [
        jax.device_put(_np.zeros((n_cores * z.shape[0], *z.shape[1:]), z.dtype), sh)
        for z in zero_outs
    ]
    outs = sharded(*concat_in, *concat_zeros)
    jax.block_until_ready(outs)
    t0 = time.time()
    for _ in range(n_iters):
        outs = sharded(*concat_in, *concat_zeros)
    jax.block_until_ready(outs)
    t1 = time.time()
    per_call_ns = (t1 - t0) / n_iters * 1e9
    results = [
        {name: _np.asarray(outs[i]).reshape(n_cores, *out_avals[i].shape)[c]
         for i, name in enumerate(out_names)}
        for c in range(n_cores)
    ]
    return results, per_call_ns


def kernel(**inputs):
    global LAST_EXEC_NS
    n_layers = int(inputs.pop('_n_layers', L))
    if n_layers not in _CACHE:
        _CACHE[n_layers] = _build(n_layers)
    nc = _CACHE[n_layers]
    g = _prep_weights(inputs, n_layers)

    x = inputs['x']
    xr = x.reshape(B, S, NI)
    in_maps = []
    for ci in range(NCORES):
        xc = xr[ci * NB:(ci + 1) * NB].astype(np.float64)  # [16, 50, 1200]
        xa = np.zeros((NB, S, NIP), np.float32)
        xa[:, :, :NI] = xc
        xa[np.arange(NB)[:, None], np.arange(S)[None, :], NI + np.arange(S)[None, :]] = 1.0
        # xT [NIP, T]: feature-major, tokens (b, s)
        xT = np.ascontiguousarray(xa.reshape(T, NIP).T)
        m = dict(g)
        m['xT'] = xT
        in_maps.append(m)

    if TRACE:
        results, per_call_ns = _run_timed(nc, in_maps)
        LAST_EXEC_NS = int(per_call_ns)
    else:
        res = run_bass_kernel_spmd(nc, in_maps, core_ids=list(range(NCORES)))
        LAST_EXEC_NS = res.exec_time_ns
        results = res.results
    outs = [r['out'].T for r in results]   # each [NB, NCLS]
    return np.concatenate(outs, axis=0).astype(np.float32)



# revision 11
# speedup vs baseline: 1.0484x; 1.0484x over previous
import sys
sys.path.insert(0, '/opt/trn_rl_repo')
import numpy as np
import math

import concourse.bass as bass
import concourse.mybir as mybir
import concourse.tile as tile
from concourse import bacc
from concourse.bass_utils import run_bass_kernel_spmd

# Problem dims
B, SL, CH, HZ = 128, 5000, 12, 100
L, D, DFF, H, NCLS = 5, 1024, 4096, 16, 71
NI = CH * HZ          # 1200
S = SL // HZ          # 50
NCORES = 8
NB = B // NCORES      # 16 batches per core
T = NB * S            # 800 tokens per core
NIP = 1280            # padded input-feature dim (1200 + 50 one-hot pos rows)
NKI = NIP // 128      # 10 input k-chunks
DK = D // H           # 64
NDC = D // 128        # 8 d-chunks
NFC = DFF // 128      # 32 ff-chunks
HB = NB // 2          # 8 batches per half
HT = HB * S           # 400 tokens per half

F32R = mybir.dt.float32r
F32 = mybir.dt.float32
BF16 = mybir.dt.bfloat16
EXP = mybir.ActivationFunctionType.Exp
RELU = mybir.ActivationFunctionType.Relu
SQRT = mybir.ActivationFunctionType.Sqrt
AOP = mybir.AluOpType

TRACE = False
LAST_EXEC_NS = None
_CACHE = {}


def _build(n_layers=L):
    nc = bacc.Bacc(None)
    d = {}
    d['xT'] = nc.dram_tensor("xT", [NIP, T], BF16, kind="ExternalInput")
    d['ones'] = nc.dram_tensor("ones", [128, 512], F32R, kind="ExternalInput")
    d['sel'] = nc.dram_tensor("sel", [1, 256], F32R, kind="ExternalInput")
    d['emb_w'] = nc.dram_tensor("emb_w", [NDC, 128, NKI * 128], BF16, kind="ExternalInput")
    d['qkv_w'] = nc.dram_tensor("qkv_w", [L, 2, NDC, 128, NDC * 128], BF16, kind="ExternalInput")
    d['wv_nat'] = nc.dram_tensor("wv_nat", [L, NDC, 128, D], BF16, kind="ExternalInput")
    d['wo_w'] = nc.dram_tensor("wo_w", [L, NDC, 128, NDC * 128], BF16, kind="ExternalInput")
    d['w1_w'] = nc.dram_tensor("w1_w", [L, NFC, 128, NDC * 128], BF16, kind="ExternalInput")
    d['w2_w'] = nc.dram_tensor("w2_w", [L, NDC, 128, NFC * 128], BF16, kind="ExternalInput")
    # bias_all cols: 0:8 q, 8:16 k, 16:24 wo, 24:56 w1, 56:64 w2
    d['bias_all'] = nc.dram_tensor("bias_all", [L, 128, 64], F32, kind="ExternalInput")
    d['cf_w'] = nc.dram_tensor("cf_w", [NDC, 128, NDC * 128], BF16, kind="ExternalInput")
    d['cf_bT'] = nc.dram_tensor("cf_bT", [128, NDC], F32, kind="ExternalInput")
    d['fc_w'] = nc.dram_tensor("fc_w", [128, NDC * NCLS], BF16, kind="ExternalInput")
    d['fc_b'] = nc.dram_tensor("fc_b", [NCLS, 1], F32, kind="ExternalInput")
    out = nc.dram_tensor("out", [NCLS, NB], F32, kind="ExternalOutput")

    with tile.TileContext(nc) as tc:
        _emit(nc, tc, d, out, n_layers)
    nc.compile()
    return nc


def _emit(nc, tc, d, out, n_layers):
    import contextlib
    ctx = contextlib.ExitStack()
    with ctx:
        sb1 = ctx.enter_context(tc.tile_pool(name="sb1", bufs=1))
        aT_p = ctx.enter_context(tc.tile_pool(name="aTp", bufs=1))
        big_p = ctx.enter_context(tc.tile_pool(name="bigp", bufs=1))
        sq_p = ctx.enter_context(tc.tile_pool(name="sqp", bufs=2))
        ws_p = ctx.enter_context(tc.tile_pool(name="wsp", bufs=6))
        wv_p = ctx.enter_context(tc.tile_pool(name="wvp", bufs=1))
        v_p = ctx.enter_context(tc.tile_pool(name="vp", bufs=1))
        pt_p = ctx.enter_context(tc.tile_pool(name="ptp", bufs=4))
        rd_p = ctx.enter_context(tc.tile_pool(name="rdp", bufs=2))
        rows_p = ctx.enter_context(tc.tile_pool(name="rowsp", bufs=4))
        bias_p = ctx.enter_context(tc.tile_pool(name="biasp", bufs=2))
        ps_mm = ctx.enter_context(tc.tile_pool(name="psmm", bufs=6, space="PSUM"))
        ps_den = ctx.enter_context(tc.tile_pool(name="psden", bufs=2, space="PSUM"))

        # persistent tiles
        hT = sb1.tile([128, NDC, T], F32R, tag="hT")
        qT = sb1.tile([128, NDC, T], BF16, tag="qT")
        kT = sb1.tile([128, NDC, T], BF16, tag="kT")
        oT = sb1.tile([128, NDC, T], BF16, tag="oT")
        ones_c = sb1.tile([128, 1], F32R, tag="ones_c")
        ones_cb = sb1.tile([128, 1], BF16, tag="ones_cb")
        ones_r = sb1.tile([1, 512], F32R, tag="ones_r")
        nc.sync.dma_start(ones_c[:], d['ones'][:, 0:1])
        nc.sync.dma_start(ones_r[:], d['ones'][0:1, :])
        nc.vector.tensor_copy(ones_cb[:], ones_c[:])
        # sel [1,256]: cols 0:128 = row-half-E selector (1s at 0:64),
        # cols 128:256 = row-half-O selector (1s at 64:128)
        sel = sb1.tile([1, 256], F32R, tag="sel")
        nc.sync.dma_start(sel[:], d['sel'][:])

        def ln_half(src, hc0, ncols, dst):
            """dst[:, c, hc0:hc0+ncols](bf16) = LN over feature dim of
            src[:, c, hc0:hc0+ncols] (fp32r, feature-major)."""
            Dn = float(NDC * 128)
            cm = 1.0 / Dn
            cv2 = 1.0 / (Dn - 1.0)
            cv1 = -1.0 / (Dn * (Dn - 1.0))
            s1 = ps_den.tile([64, ncols], F32, tag="den")
            s2 = ps_den.tile([64, ncols], F32, tag="den")
            for c in range(NDC):
                sq = sq_p.tile([128, HT], F32R, tag="sq")
                nc.vector.tensor_tensor(out=sq[:, 0:ncols], in0=src[:, c, hc0:hc0 + ncols],
                                        in1=src[:, c, hc0:hc0 + ncols], op=AOP.mult)
                nc.tensor.matmul(s1[0:1, :], ones_c[:], src[:, c, hc0:hc0 + ncols],
                                 start=(c == 0), stop=(c == NDC - 1))
                nc.tensor.matmul(s2[0:1, :], ones_c[:], sq[:, 0:ncols],
                                 start=(c == 0), stop=(c == NDC - 1))
            m_row = rows_p.tile([1, HT], F32R, tag="rows")
            t1 = rows_p.tile([1, HT], F32, tag="rows")
            tv = rows_p.tile([1, HT], F32, tag="rows")
            r_row = rows_p.tile([1, HT], F32R, tag="rows")
            nc.vector.tensor_scalar_mul(m_row[:, 0:ncols], s1[0:1, :], cm)
            # cv1*s1^2 == (cv1*Dn*Dn) * m_row^2  (m_row is SBUF; s1 is PSUM)
            nc.vector.scalar_tensor_tensor(out=t1[:, 0:ncols], in0=m_row[:, 0:ncols],
                                           scalar=cv1 * Dn * Dn, in1=m_row[:, 0:ncols],
                                           op0=mybir.AluOpType.mult,
                                           op1=mybir.AluOpType.mult)
            nc.vector.scalar_tensor_tensor(out=tv[:, 0:ncols], in0=s2[0:1, :], scalar=cv2,
                                           in1=t1[:, 0:ncols], op0=mybir.AluOpType.mult,
                                           op1=mybir.AluOpType.add)
            nc.scalar.activation(tv[:, 0:ncols], tv[:, 0:ncols], SQRT, bias=0.0, scale=1.0)
            nc.vector.tensor_scalar_add(tv[:, 0:ncols], tv[:, 0:ncols], 1e-6)
            with nc.allow_low_precision(reason="fp32r rounding of 1/(std+eps)"):
                nc.vector.reciprocal(r_row[:, 0:ncols], tv[:, 0:ncols])
            Mb = ps_mm.tile([128, HT], F32, tag="mm")
            Rb = ps_mm.tile([128, HT], F32, tag="mm")
            nc.tensor.matmul(Mb[:, 0:ncols], ones_r[0:1, 0:128], m_row[:, 0:ncols],
                             start=True, stop=True)
            nc.tensor.matmul(Rb[:, 0:ncols], ones_r[0:1, 0:128], r_row[:, 0:ncols],
                             start=True, stop=True)
            for c in range(NDC):
                tmp = sq_p.tile([128, HT], F32R, tag="sq")
                nc.vector.tensor_tensor(out=tmp[:, 0:ncols], in0=src[:, c, hc0:hc0 + ncols],
                                        in1=Mb[:, 0:ncols], op=AOP.subtract)
                nc.vector.tensor_tensor(out=dst[:, c, hc0:hc0 + ncols], in0=tmp[:, 0:ncols],
                                        in1=Rb[:, 0:ncols], op=AOP.mult)

        # ---------------- embed ----------------
        xt = big_p.tile([128, NKI, T], BF16, tag="big")
        nc.sync.dma_start(xt[:], d['xT'].rearrange("(k p) t -> p k t", p=128))
        for m in range(NDC):
            wtA = ws_p.tile([128, NDC, 128], BF16, tag="ws")
            wtB = ws_p.tile([128, NDC, 128], BF16, tag="ws")
            emb_ap = d['emb_w'][m].rearrange("p (k c) -> p k c", k=NKI)
            nc.sync.dma_start(wtA[:], emb_ap[:, 0:NDC, :])
            nc.sync.dma_start(wtB[:, 0:NKI - NDC, :], emb_ap[:, NDC:NKI, :])
            ps0 = ps_mm.tile([128, HT], F32, tag="mm")
            ps1 = ps_mm.tile([128, HT], F32, tag="mm")
            for k in range(NKI):
                wt = wtA[:, k, :] if k < NDC else wtB[:, k - NDC, :]
                nc.tensor.matmul(ps0[:], wt, xt[:, k, 0:HT],
                                 start=(k == 0), stop=(k == NKI - 1))
                nc.tensor.matmul(ps1[:], wt, xt[:, k, HT:T],
                                 start=(k == 0), stop=(k == NKI - 1))
            nc.vector.tensor_copy(hT[:, m, 0:HT], ps0[:])
            nc.vector.tensor_copy(hT[:, m, HT:T], ps1[:])

        # ---------------- layers ----------------
        for li in range(n_layers):
            last = (li == n_layers - 1) and (n_layers == L)
            bia = bias_p.tile([128, 64], F32, tag="bias")
            nc.sync.dma_start(bia[:], d['bias_all'][li])
            aT = aT_p.tile([128, NDC, T], BF16, tag="aT")
            ln_half(hT, 0, HT, aT)
            ln_half(hT, HT, HT, aT)
            # ---- Q, K (full T, weights loaded once) ----
            for mat, dst in ((0, qT), (1, kT)):
                for m in range(NDC):
                    wt = ws_p.tile([128, NDC, 128], BF16, tag="ws")
                    nc.sync.dma_start(wt[:],
                                      d['qkv_w'][li, mat, m].rearrange("p (k c) -> p k c", k=NDC))
                    ps0 = ps_mm.tile([128, HT], F32, tag="mm")
                    ps1 = ps_mm.tile([128, HT], F32, tag="mm")
                    for k in range(NDC):
                        nc.tensor.matmul(ps0[:], wt[:, k, :], aT[:, k, 0:HT],
                                         start=(k == 0), stop=(k == NDC - 1))
                        nc.tensor.matmul(ps1[:], wt[:, k, :], aT[:, k, HT:T],
                                         start=(k == 0), stop=(k == NDC - 1))
                    bcol = bia[:, mat * NDC + m:mat * NDC + m + 1]
                    nc.vector.tensor_scalar_add(dst[:, m, 0:HT], ps0[:], bcol)
                    nc.vector.tensor_scalar_add(dst[:, m, HT:T], ps1[:], bcol)
            # ---- V weights (resident for the layer) ----
            wv = wv_p.tile([128, NDC, D], BF16, tag="wv")
            nc.sync.dma_start(wv[:], d['wv_nat'][li].rearrange("k p n -> p k n"))
            for hf in range(2):
                hc0 = hf * HT
                # V (token-major, per batch)
                v = v_p.tile([64, HB, D], BF16, tag="v")
                for bi in range(HB):
                    bc0 = hc0 + bi * S
                    psv0 = ps_mm.tile([64, 512], F32, tag="mm")
                    psv1 = ps_mm.tile([64, 512], F32, tag="mm")
                    for k in range(NDC):
                        nc.tensor.matmul(psv0[0:S, :], aT[:, k, bc0:bc0 + S],
                                         wv[:, k, 0:512], start=(k == 0), stop=(k == NDC - 1))
                        nc.tensor.matmul(psv1[0:S, :], aT[:, k, bc0:bc0 + S],
                                         wv[:, k, 512:1024], start=(k == 0), stop=(k == NDC - 1))
                    nc.scalar.copy(v[0:S, bi, 0:512], psv0[0:S, :])
                    nc.scalar.copy(v[0:S, bi, 512:1024], psv1[0:S, :])
                # attention per batch
                for bi in range(HB):
                    bc0 = hc0 + bi * S
                    psE = ps_mm.tile([64, 8 * S], F32, tag="mm")
                    psO = ps_mm.tile([64, 8 * S], F32, tag="mm")
                    for c in range(NDC):
                        nc.tensor.matmul(psE[0:S, c * S:(c + 1) * S],
                                         kT[0:DK, c, bc0:bc0 + S], qT[0:DK, c, bc0:bc0 + S],
                                         start=True, stop=True)
                    for c in range(NDC):
                        nc.tensor.matmul(psO[0:S, c * S:(c + 1) * S],
                                         kT[DK:128, c, bc0:bc0 + S], qT[DK:128, c, bc0:bc0 + S],
                                         start=True, stop=True)
                    pTE = pt_p.tile([64, 8 * S], BF16, tag="pt")
                    pTO = pt_p.tile([64, 8 * S], BF16, tag="pt")
                    nc.scalar.activation(pTE[0:S, :], psE[0:S, :], EXP,
                                         bias=0.0, scale=1.0 / math.sqrt(DK))
                    nc.scalar.activation(pTO[0:S, :], psO[0:S, :], EXP,
                                         bias=0.0, scale=1.0 / math.sqrt(DK))
                    denE = ps_den.tile([64, 8 * S], F32, tag="den")
                    denO = ps_den.tile([64, 8 * S], F32, tag="den")
                    nc.tensor.matmul(denE[0:1, :], ones_cb[0:S, :], pTE[0:S, :],
                                     start=True, stop=True)
                    nc.tensor.matmul(denO[0:1, :], ones_cb[0:S, :], pTO[0:S, :],
                                     start=True, stop=True)
                    rd = rd_p.tile([1, 16 * S], F32R, tag="rd")
                    with nc.allow_low_precision(reason="softmax denom reciprocal"):
                        nc.vector.reciprocal(rd[:, 0:8 * S], denE[0:1, :])
                        nc.vector.reciprocal(rd[:, 8 * S:16 * S], denO[0:1, :])
                    rdB = ps_mm.tile([128, 8 * S], F32, tag="mm")
                    nc.tensor.matmul(rdB[:], sel[:, 0:128], rd[:, 0:8 * S],
                                     start=True, stop=False)
                    nc.tensor.matmul(rdB[:], sel[:, 128:256], rd[:, 8 * S:16 * S],
                                     start=False, stop=True)
                    rdS = sq_p.tile([128, 8 * S], F32R, tag="rds")
                    nc.scalar.copy(rdS[:], rdB[:])
                    po = ps_mm.tile([128, 8 * S], F32, tag="mm")
                    for c in range(NDC):
                        nc.tensor.matmul(po[0:DK, c * S:(c + 1) * S],
                                         v[0:S, bi, (2 * c) * DK:(2 * c + 1) * DK],
                                         pTE[0:S, c * S:(c + 1) * S], start=True, stop=True)
                    for c in range(NDC):
                        nc.tensor.matmul(po[DK:128, c * S:(c + 1) * S],
                                         v[0:S, bi, (2 * c + 1) * DK:(2 * c + 2) * DK],
                                         pTO[0:S, c * S:(c + 1) * S], start=True, stop=True)
                    nc.vector.tensor_tensor(
                        out=oT[:, :, bc0:bc0 + S],
                        in0=po[:].rearrange("p (c t) -> p c t", c=NDC),
                        in1=rdS[:].rearrange("p (c t) -> p c t", c=NDC),
                        op=AOP.mult)
            # ---- Wo + residual ----
            for m in range(NDC):
                wt = ws_p.tile([128, NDC, 128], BF16, tag="ws")
                nc.sync.dma_start(wt[:],
                                  d['wo_w'][li, m].rearrange("p (k c) -> p k c", k=NDC))
                ps0 = ps_mm.tile([128, HT], F32, tag="mm")
                ps1 = ps_mm.tile([128, HT], F32, tag="mm")
                for k in range(NDC):
                    nc.tensor.matmul(ps0[:], wt[:, k, :], oT[:, k, 0:HT],
                                     start=(k == 0), stop=(k == NDC - 1))
                    nc.tensor.matmul(ps1[:], wt[:, k, :], oT[:, k, HT:T],
                                     start=(k == 0), stop=(k == NDC - 1))
                bcol = bia[:, 16 + m:16 + m + 1]
                nc.vector.scalar_tensor_tensor(out=hT[:, m, 0:HT], in0=ps0[:], scalar=bcol,
                                               in1=hT[:, m, 0:HT], op0=AOP.add, op1=AOP.add)
                nc.vector.scalar_tensor_tensor(out=hT[:, m, HT:T], in0=ps1[:], scalar=bcol,
                                               in1=hT[:, m, HT:T], op0=AOP.add, op1=AOP.add)
            # ---- FFN ----
            if not last:
                aT2 = aT_p.tile([128, NDC, T], BF16, tag="aT")
                ln_half(hT, 0, HT, aT2)
                ln_half(hT, HT, HT, aT2)
                ffq = big_p.tile([128, NFC, T], BF16, tag="big")
                for m in range(NFC):
                    wt = ws_p.tile([128, NDC, 128], BF16, tag="ws")
                    nc.sync.dma_start(wt[:],
                                      d['w1_w'][li, m].rearrange("p (k c) -> p k c", k=NDC))
                    ps0 = ps_mm.tile([128, HT], F32, tag="mm")
                    ps1 = ps_mm.tile([128, HT], F32, tag="mm")
                    for k in range(NDC):
                        nc.tensor.matmul(ps0[:], wt[:, k, :], aT2[:, k, 0:HT],
                                         start=(k == 0), stop=(k == NDC - 1))
                        nc.tensor.matmul(ps1[:], wt[:, k, :], aT2[:, k, HT:T],
                                         start=(k == 0), stop=(k == NDC - 1))
                    bcol = bia[:, 24 + m:24 + m + 1]
                    nc.vector.tensor_scalar(out=ffq[:, m, 0:HT], in0=ps0[:], scalar1=bcol,
                                            scalar2=0.0, op0=AOP.add, op1=AOP.max)
                    nc.vector.tensor_scalar(out=ffq[:, m, HT:T], in0=ps1[:], scalar1=bcol,
                                            scalar2=0.0, op0=AOP.add, op1=AOP.max)
                for m in range(NDC):
                    w2ap = d['w2_w'][li, m].rearrange("p (k c) -> p k c", k=NFC)
                    w2ts = []
                    for kb in range(4):
                        w2t = ws_p.tile([128, NDC, 128], BF16, tag="ws")
                        nc.sync.dma_start(w2t[:], w2ap[:, kb * NDC:(kb + 1) * NDC, :])
                        w2ts.append(w2t)
                    ps0 = ps_mm.tile([128, HT], F32, tag="mm")
                    ps1 = ps_mm.tile([128, HT], F32, tag="mm")
                    for k in range(NFC):
                        w2t = w2ts[k // NDC]
                        nc.tensor.matmul(ps0[:], w2t[:, k % NDC, :], ffq[:, k, 0:HT],
                                         start=(k == 0), stop=(k == NFC - 1))
                        nc.tensor.matmul(ps1[:], w2t[:, k % NDC, :], ffq[:, k, HT:T],
                                         start=(k == 0), stop=(k == NFC - 1))
                    bcol = bia[:, 56 + m:56 + m + 1]
                    nc.vector.scalar_tensor_tensor(out=hT[:, m, 0:HT], in0=ps0[:], scalar=bcol,
                                                   in1=hT[:, m, 0:HT], op0=AOP.add, op1=AOP.add)
                    nc.vector.scalar_tensor_tensor(out=hT[:, m, HT:T], in0=ps1[:], scalar=bcol,
                                                   in1=hT[:, m, HT:T], op0=AOP.add, op1=AOP.add)
            else:
                # last layer: FFN only for the last token of each batch
                hL = sb1.tile([128, NDC, NB], F32R, tag="hL")
                for c in range(NDC):
                    nc.vector.tensor_copy(
                        hL[:, c, :],
                        hT[:, c, :].rearrange("p (b s) -> p b s", s=S)[:, :, S - 1])
                aL = sb1.tile([128, NDC, NB], BF16, tag="aL")
                ln_half(hL, 0, NB, aL)
                ffL = sb1.tile([128, NFC, NB], BF16, tag="ffL")
                for m in range(NFC):
                    wt = ws_p.tile([128, NDC, 128], BF16, tag="ws")
                    nc.sync.dma_start(wt[:],
                                      d['w1_w'][li, m].rearrange("p (k c) -> p k c", k=NDC))
                    ps = ps_mm.tile([128, HT], F32, tag="mm")
                    for k in range(NDC):
                        nc.tensor.matmul(ps[:, 0:NB], wt[:, k, :], aL[:, k, :],
                                         start=(k == 0), stop=(k == NDC - 1))
                    bcol = bia[:, 24 + m:24 + m + 1]
                    nc.vector.tensor_scalar(out=ffL[:, m, :], in0=ps[:, 0:NB], scalar1=bcol,
                                            scalar2=0.0, op0=AOP.add, op1=AOP.max)
                for m in range(NDC):
                    w2ap = d['w2_w'][li, m].rearrange("p (k c) -> p k c", k=NFC)
                    w2ts = []
                    for kb in range(4):
                        w2t = ws_p.tile([128, NDC, 128], BF16, tag="ws")
                        nc.sync.dma_start(w2t[:], w2ap[:, kb * NDC:(kb + 1) * NDC, :])
                        w2ts.append(w2t)
                    ps = ps_mm.tile([128, HT], F32, tag="mm")
                    for k in range(NFC):
                        nc.tensor.matmul(ps[:, 0:NB], w2ts[k // NDC][:, k % NDC, :], ffL[:, k, :],
                                         start=(k == 0), stop=(k == NFC - 1))
                    bcol = bia[:, 56 + m:56 + m + 1]
                    nc.vector.scalar_tensor_tensor(out=hL[:, m, :], in0=ps[:, 0:NB], scalar=bcol,
                                                   in1=hL[:, m, :], op0=AOP.add, op1=AOP.add)

        # ---------------- head ----------------
        if n_layers == L:
            src_pool = hL
        else:
            src_pool = sb1.tile([128, NDC, NB], F32R, tag="hL")
            for c in range(NDC):
                nc.vector.tensor_copy(
                    src_pool[:, c, :],
                    hT[:, c, :].rearrange("p (b s) -> p b s", s=S)[:, :, S - 1])
        pL = sb1.tile([128, NDC, NB], BF16, tag="pL")
        ln_half(src_pool, 0, NB, pL)
        cbT = bias_p.tile([128, 64], F32, tag="bias")
        nc.sync.dma_start(cbT[:, 0:NDC], d['cf_bT'][:])
        z1 = sb1.tile([128, NDC, NB], BF16, tag="z1")
        for m in range(NDC):
            wt = ws_p.tile([128, NDC, 128], BF16, tag="ws")
            nc.sync.dma_start(wt[:], d['cf_w'][m].rearrange("p (k c) -> p k c", k=NDC))
            ps = ps_mm.tile([128, HT], F32, tag="mm")
            for k in range(NDC):
                nc.tensor.matmul(ps[:, 0:NB], wt[:, k, :], pL[:, k, :],
                                 start=(k == 0), stop=(k == NDC - 1))
            nc.vector.tensor_scalar(out=z1[:, m, :], in0=ps[:, 0:NB], scalar1=cbT[:, m:m + 1],
                                    scalar2=0.0, op0=AOP.add, op1=AOP.max)
        fwt = sb1.tile([128, NDC, NCLS], BF16, tag="fwt")
        nc.sync.dma_start(fwt[:], d['fc_w'].rearrange("p (k c) -> p k c", k=NDC))
        fb = rows_p.tile([NCLS, 1], F32, tag="fb")
        nc.sync.dma_start(fb[:], d['fc_b'][:])
        ps = ps_mm.tile([128, HT], F32, tag="mm")
        for k in range(NDC):
            nc.tensor.matmul(ps[0:NCLS, 0:NB], fwt[:, k, :], z1[:, k, :],
                             start=(k == 0), stop=(k == NDC - 1))
        osb = sb1.tile([NCLS, NB], F32, tag="osb")
        nc.vector.tensor_scalar_add(osb[:], ps[0:NCLS, 0:NB], fb[:])
        nc.sync.dma_start(out[:], osb[:])


def _prep_weights(inputs, n_layers=L):
    import ml_dtypes
    f64 = np.float64
    bf16 = ml_dtypes.bfloat16

    def prep_lhsT(W):
        # W [K, M] -> [M/128, 128, (K/128)*128] : tile[p, k*128+c] = W[k*128+p, mb*128+c]
        K, M = W.shape
        nk, nm = K // 128, M // 128
        return np.ascontiguousarray(
            W.reshape(nk, 128, nm, 128).transpose(2, 1, 0, 3).reshape(nm, 128, nk * 128)
        ).astype(bf16)

    emb = inputs['embed_w'].astype(f64)          # [1200, 1024]
    pos = np.arange(S, dtype=f64)[:, None]
    div = np.exp(np.arange(0, D, 2, dtype=np.float32).astype(f64) * (-math.log(10000.0) / D))
    pe = np.zeros((S, D), f64)
    pe[:, 0::2] = np.sin(pos * div)
    pe[:, 1::2] = np.cos(pos * div)
    Wp = np.zeros((NIP, D), f64)
    Wp[:NI] = emb
    Wp[NI:NI + S] = pe
    g = {}
    g['emb_w'] = prep_lhsT(Wp)

    ln_g = inputs['ln_g'].astype(f64); ln_b = inputs['ln_b'].astype(f64)
    aw = inputs['attn_w'].astype(f64); ab = inputs['attn_b'].astype(f64)
    fw1 = inputs['ff_w1'].astype(f64); fb1 = inputs['ff_b1'].astype(f64)
    fw2 = inputs['ff_w2'].astype(f64); fb2 = inputs['ff_b2'].astype(f64)

    qkv_w = np.zeros((L, 2, NDC, 128, NDC * 128), bf16)
    wv_nat = np.zeros((L, NDC, 128, D), bf16)
    wo_w = np.zeros((L, NDC, 128, NDC * 128), bf16)
    w1_w = np.zeros((L, NFC, 128, NDC * 128), bf16)
    w2_w = np.zeros((L, NDC, 128, NFC * 128), bf16)
    bias_all = np.zeros((L, 128, 64), np.float32)

    for i in range(n_layers):
        g1, b1 = ln_g[i, 0][:, None], ln_b[i, 0]
        for mat in range(3):
            We = g1 * aw[i, mat]
            be = ab[i, mat] + b1 @ aw[i, mat]
            if mat == 2:
                wv_nat[i] = We.astype(bf16).reshape(NDC, 128, D)
                bv = be  # v bias folded into wo bias below (softmax rows sum to 1)
            else:
                qkv_w[i, mat] = prep_lhsT(We)
                bias_all[i, :, mat * NDC:(mat + 1) * NDC] = be.reshape(NDC, 128).T
        wo_w[i] = prep_lhsT(aw[i, 3])
        wo_be = ab[i, 3] + bv @ aw[i, 3]
        bias_all[i, :, 16:24] = wo_be.reshape(NDC, 128).T
        g2, b2 = ln_g[i, 1][:, None], ln_b[i, 1]
        W1e = g2 * fw1[i]
        b1e = fb1[i] + b2 @ fw1[i]
        w1_w[i] = prep_lhsT(W1e)
        bias_all[i, :, 24:56] = b1e.reshape(NFC, 128).T
        w2_w[i] = prep_lhsT(fw2[i])
        bias_all[i, :, 56:64] = fb2[i].reshape(NDC, 128).T

    g['qkv_w'] = qkv_w; g['wv_nat'] = wv_nat; g['wo_w'] = wo_w
    g['w1_w'] = w1_w; g['w2_w'] = w2_w; g['bias_all'] = bias_all

    inv = 1.0 / math.sqrt(1.0 + 1e-5)
    fin_g = inputs['fin_g'].astype(f64); fin_b = inputs['fin_b'].astype(f64)
    A1 = fin_g * inv * inputs['cf_bn_g'].astype(f64)
    C1 = fin_b * inv * inputs['cf_bn_g'].astype(f64) + inputs['cf_bn_b'].astype(f64)
    cfw = inputs['cf_w'].astype(f64)
    cf_we = A1[:, None] * cfw
    cf_be = inputs['cf_b'].astype(f64) + C1 @ cfw
    g['cf_w'] = prep_lhsT(cf_we)
    g['cf_bT'] = cf_be.reshape(NDC, 128).T.astype(np.float32)
    A2 = inv * inputs['fc_bn_g'].astype(f64)
    C2 = inputs['fc_bn_b'].astype(f64)
    fcw = inputs['fc_w'].astype(f64)
    fc_we = A2[:, None] * fcw
    fc_be = inputs['fc_b'].astype(f64) + C2 @ fcw
    g['fc_w'] = np.ascontiguousarray(
        fc_we.reshape(NDC, 128, NCLS).transpose(1, 0, 2).reshape(128, NDC * NCLS)
    ).astype(bf16)
    g['fc_b'] = fc_be.reshape(NCLS, 1).astype(np.float32)
    g['ones'] = np.ones((128, 512), np.float32)
    selm = np.zeros((1, 256), np.float32)
    selm[0, 0:64] = 1.0
    selm[0, 192:256] = 1.0
    g['sel'] = selm
    return g


def _run_timed(nc, in_maps, n_iters=10):
    """Mirror bass2jax.run_bass_via_pjrt (no donation), time steady-state execs."""
    import time
    import jax
    import numpy as _np
    from jax.experimental.shard_map import shard_map
    from jax.sharding import Mesh, PartitionSpec, NamedSharding
    from concourse import bass2jax as b2j
    from concourse import mybir as _mb

    b2j.install_neuronx_cc_hook()
    n_cores = len(in_maps)
    partition_name = nc.partition_id_tensor.name if nc.partition_id_tensor else None
    in_names, out_names, out_avals, zero_outs = [], [], [], []
    for alloc in nc.m.functions[0].allocations:
        if not isinstance(alloc, _mb.MemoryLocationSet):
            continue
        name = alloc.memorylocations[0].name
        if alloc.kind == "ExternalInput":
            if name != partition_name:
                in_names.append(name)
        elif alloc.kind == "ExternalOutput":
            shape = tuple(alloc.tensor_shape)
            dtype = _mb.dt.np(alloc.dtype)
            out_names.append(name)
            out_avals.append(jax.core.ShapedArray(shape, dtype))
            zero_outs.append(_np.zeros(shape, dtype))
    n_params = len(in_names)
    all_in_names = list(in_names) + list(out_names)
    if partition_name is not None:
        all_in_names.append(partition_name)

    def _body(*args):
        operands = list(args)
        if partition_name is not None:
            operands.append(b2j.partition_id_tensor())
        outs = b2j._bass_exec_p.bind(
            *operands,
            out_avals=tuple(out_avals),
            in_names=tuple(all_in_names),
            out_names=tuple(out_names),
            lowering_input_output_aliases=(),
            sim_require_finite=True,
            sim_require_nnan=True,
            nc=nc,
        )
        return tuple(outs)

    devices = jax.devices()[:n_cores]
    mesh = Mesh(_np.asarray(devices), ("core",))
    spec = PartitionSpec("core")
    sharded = jax.jit(shard_map(
        _body, mesh=mesh, in_specs=(spec,) * (n_params + len(out_names)),
        out_specs=(spec,) * len(out_names), check_rep=False))
    sh = NamedSharding(mesh, spec)
    concat_in = [
        jax.device_put(_np.concatenate([_np.asarray(m[name]) for m in in_maps], axis=0), sh)
        for name in in_names
    ]
    concat_zeros = [
        jax.device_put(_np.zeros((n_cores * z.shape[0], *z.shape[1:]), z.dtype), sh)
        for z in zero_outs
    ]
    outs = sharded(*concat_in, *concat_zeros)
    jax.block_until_ready(outs)
    t0 = time.time()
    for _ in range(n_iters):
        outs = sharded(*concat_in, *concat_zeros)
    jax.block_until_ready(outs)
    t1 = time.time()
    per_call_ns = (t1 - t0) / n_iters * 1e9
    results = [
        {name: _np.asarray(outs[i]).reshape(n_cores, *out_avals[i].shape)[c]
         for i, name in enumerate(out_names)}
        for c in range(n_cores)
    ]
    return results, per_call_ns


def kernel(**inputs):
    global LAST_EXEC_NS
    import ml_dtypes
    bf16 = ml_dtypes.bfloat16
    n_layers = int(inputs.pop('_n_layers', L))
    if n_layers not in _CACHE:
        _CACHE[n_layers] = _build(n_layers)
    nc = _CACHE[n_layers]
    g = _prep_weights(inputs, n_layers)

    x = inputs['x']
    xr = x.reshape(B, S, NI)
    in_maps = []
    for ci in range(NCORES):
        xc = xr[ci * NB:(ci + 1) * NB].astype(np.float32)  # [16, 50, 1200]
        xa = np.zeros((NB, S, NIP), np.float32)
        xa[:, :, :NI] = xc
        xa[np.arange(NB)[:, None], np.arange(S)[None, :], NI + np.arange(S)[None, :]] = 1.0
        # xT [NIP, T]: feature-major, tokens (b, s)
        xT = np.ascontiguousarray(xa.reshape(T, NIP).T).astype(bf16)
        m = dict(g)
        m['xT'] = xT
        in_maps.append(m)

    if TRACE:
        results, per_call_ns = _run_timed(nc, in_maps)
        LAST_EXEC_NS = int(per_call_ns)
    else:
        res = run_bass_kernel_spmd(nc, in_maps, core_ids=list(range(NCORES)))
        LAST_EXEC_NS = res.exec_time_ns
        results = res.results
    outs = [r['out'].T for r in results]   # each [NB, NCLS]
    return np.concatenate(outs, axis=0).astype(np.float32)


# revision 12
# speedup vs baseline: 1.1247x; 1.0728x over previous
import sys
sys.path.insert(0, '/opt/trn_rl_repo')
import numpy as np
import math

import concourse.bass as bass
import concourse.mybir as mybir
import concourse.tile as tile
from concourse import bacc
from concourse.bass_utils import run_bass_kernel_spmd

# Problem dims
B, SL, CH, HZ = 128, 5000, 12, 100
L, D, DFF, H, NCLS = 5, 1024, 4096, 16, 71
NI = CH * HZ          # 1200
S = SL // HZ          # 50
NCORES = 8
NB = B // NCORES      # 16 batches per core
T = NB * S            # 800 tokens per core
NIP = 1280            # padded input-feature dim (1200 + 50 one-hot pos rows)
NKI = NIP // 128      # 10 input k-chunks
DK = D // H           # 64
NDC = D // 128        # 8 d-chunks
NFC = DFF // 128      # 32 ff-chunks
HB = NB // 2          # 8 batches per half
HT = HB * S           # 400 tokens per half

F32R = mybir.dt.float32r
F32 = mybir.dt.float32
BF16 = mybir.dt.bfloat16
EXP = mybir.ActivationFunctionType.Exp
RELU = mybir.ActivationFunctionType.Relu
SQRT = mybir.ActivationFunctionType.Sqrt
AOP = mybir.AluOpType

TRACE = False
LAST_EXEC_NS = None
_CACHE = {}


def _build(n_layers=L):
    nc = bacc.Bacc(None)
    d = {}
    d['xT'] = nc.dram_tensor("xT", [NIP, T], BF16, kind="ExternalInput")
    d['ones'] = nc.dram_tensor("ones", [128, 512], F32R, kind="ExternalInput")
    d['sel'] = nc.dram_tensor("sel", [1, 256], F32R, kind="ExternalInput")
    d['emb_w'] = nc.dram_tensor("emb_w", [NDC, 128, NKI * 128], BF16, kind="ExternalInput")
    d['qkv_w'] = nc.dram_tensor("qkv_w", [L, 2, NDC, 128, NDC * 128], BF16, kind="ExternalInput")
    d['wv_nat'] = nc.dram_tensor("wv_nat", [L, NDC, 128, D], BF16, kind="ExternalInput")
    d['wo_w'] = nc.dram_tensor("wo_w", [L, NDC, 128, NDC * 128], BF16, kind="ExternalInput")
    d['w1_w'] = nc.dram_tensor("w1_w", [L, NFC, 128, NDC * 128], BF16, kind="ExternalInput")
    d['w2_w'] = nc.dram_tensor("w2_w", [L, NDC, 128, NFC * 128], BF16, kind="ExternalInput")
    # bias_all cols: 0:8 q, 8:16 k, 16:24 wo, 24:56 w1, 56:64 w2
    d['bias_all'] = nc.dram_tensor("bias_all", [L, 128, 64], F32, kind="ExternalInput")
    d['cf_w'] = nc.dram_tensor("cf_w", [NDC, 128, NDC * 128], BF16, kind="ExternalInput")
    d['cf_bT'] = nc.dram_tensor("cf_bT", [128, NDC], F32, kind="ExternalInput")
    d['fc_w'] = nc.dram_tensor("fc_w", [128, NDC * NCLS], BF16, kind="ExternalInput")
    d['fc_b'] = nc.dram_tensor("fc_b", [NCLS, 1], F32, kind="ExternalInput")
    out = nc.dram_tensor("out", [NCLS, NB], F32, kind="ExternalOutput")

    with tile.TileContext(nc) as tc:
        _emit(nc, tc, d, out, n_layers)
    nc.compile()
    return nc


def _emit(nc, tc, d, out, n_layers):
    import contextlib
    ctx = contextlib.ExitStack()
    with ctx:
        sb1 = ctx.enter_context(tc.tile_pool(name="sb1", bufs=1))
        aT_p = ctx.enter_context(tc.tile_pool(name="aTp", bufs=1))
        big_p = ctx.enter_context(tc.tile_pool(name="bigp", bufs=1))
        sq_p = ctx.enter_context(tc.tile_pool(name="sqp", bufs=2))
        ws_p = ctx.enter_context(tc.tile_pool(name="wsp", bufs=6))
        wv_p = ctx.enter_context(tc.tile_pool(name="wvp", bufs=1))
        v_p = ctx.enter_context(tc.tile_pool(name="vp", bufs=1))
        pt_p = ctx.enter_context(tc.tile_pool(name="ptp", bufs=4))
        rd_p = ctx.enter_context(tc.tile_pool(name="rdp", bufs=2))
        rows_p = ctx.enter_context(tc.tile_pool(name="rowsp", bufs=4))
        bias_p = ctx.enter_context(tc.tile_pool(name="biasp", bufs=2))
        ps_mm = ctx.enter_context(tc.tile_pool(name="psmm", bufs=6, space="PSUM"))
        ps_den = ctx.enter_context(tc.tile_pool(name="psden", bufs=2, space="PSUM"))

        # persistent tiles
        hT = sb1.tile([128, NDC, T], F32R, tag="hT")
        qT = sb1.tile([128, NDC, T], BF16, tag="qT")
        kT = sb1.tile([128, NDC, T], BF16, tag="kT")
        oT = sb1.tile([128, NDC, T], BF16, tag="oT")
        ones_c = sb1.tile([128, 1], F32R, tag="ones_c")
        ones_cb = sb1.tile([128, 1], BF16, tag="ones_cb")
        ones_r = sb1.tile([1, 512], F32R, tag="ones_r")
        nc.sync.dma_start(ones_c[:], d['ones'][:, 0:1])
        nc.sync.dma_start(ones_r[:], d['ones'][0:1, :])
        nc.vector.tensor_copy(ones_cb[:], ones_c[:])
        # sel [1,256]: cols 0:128 = row-half-E selector (1s at 0:64),
        # cols 128:256 = row-half-O selector (1s at 64:128)
        sel = sb1.tile([1, 256], F32R, tag="sel")
        nc.sync.dma_start(sel[:], d['sel'][:])

        def ln_half(src, hc0, ncols, dst):
            """dst[:, c, hc0:hc0+ncols](bf16) = LN over feature dim of
            src[:, c, hc0:hc0+ncols] (fp32r, feature-major)."""
            Dn = float(NDC * 128)
            cm = 1.0 / Dn
            cv2 = 1.0 / (Dn - 1.0)
            cv1 = -1.0 / (Dn * (Dn - 1.0))
            s1 = ps_den.tile([64, ncols], F32, tag="den")
            s2 = ps_den.tile([64, ncols], F32, tag="den")
            for c in range(NDC):
                sq = sq_p.tile([128, HT], F32R, tag="sq")
                nc.vector.tensor_tensor(out=sq[:, 0:ncols], in0=src[:, c, hc0:hc0 + ncols],
                                        in1=src[:, c, hc0:hc0 + ncols], op=AOP.mult)
                nc.tensor.matmul(s1[0:1, :], ones_c[:], src[:, c, hc0:hc0 + ncols],
                                 start=(c == 0), stop=(c == NDC - 1))
                nc.tensor.matmul(s2[0:1, :], ones_c[:], sq[:, 0:ncols],
                                 start=(c == 0), stop=(c == NDC - 1))
            m_row = rows_p.tile([1, HT], F32R, tag="rows")
            t1 = rows_p.tile([1, HT], F32, tag="rows")
            tv = rows_p.tile([1, HT], F32, tag="rows")
            r_row = rows_p.tile([1, HT], F32R, tag="rows")
            nc.vector.tensor_scalar_mul(m_row[:, 0:ncols], s1[0:1, :], cm)
            # cv1*s1^2 == (cv1*Dn*Dn) * m_row^2  (m_row is SBUF; s1 is PSUM)
            nc.vector.scalar_tensor_tensor(out=t1[:, 0:ncols], in0=m_row[:, 0:ncols],
                                           scalar=cv1 * Dn * Dn, in1=m_row[:, 0:ncols],
                                           op0=mybir.AluOpType.mult,
                                           op1=mybir.AluOpType.mult)
            nc.vector.scalar_tensor_tensor(out=tv[:, 0:ncols], in0=s2[0:1, :], scalar=cv2,
                                           in1=t1[:, 0:ncols], op0=mybir.AluOpType.mult,
                                           op1=mybir.AluOpType.add)
            nc.scalar.activation(tv[:, 0:ncols], tv[:, 0:ncols], SQRT, bias=0.0, scale=1.0)
            nc.vector.tensor_scalar_add(tv[:, 0:ncols], tv[:, 0:ncols], 1e-6)
            with nc.allow_low_precision(reason="fp32r rounding of 1/(std+eps)"):
                nc.vector.reciprocal(r_row[:, 0:ncols], tv[:, 0:ncols])
            Mb = ps_mm.tile([128, HT], F32, tag="mm")
            Rb = ps_mm.tile([128, HT], F32, tag="mm")
            nc.tensor.matmul(Mb[:, 0:ncols], ones_r[0:1, 0:128], m_row[:, 0:ncols],
                             start=True, stop=True)
            nc.tensor.matmul(Rb[:, 0:ncols], ones_r[0:1, 0:128], r_row[:, 0:ncols],
                             start=True, stop=True)
            for c in range(NDC):
                tmp = sq_p.tile([128, HT], F32R, tag="sq")
                nc.vector.tensor_tensor(out=tmp[:, 0:ncols], in0=src[:, c, hc0:hc0 + ncols],
                                        in1=Mb[:, 0:ncols], op=AOP.subtract)
                nc.vector.tensor_tensor(out=dst[:, c, hc0:hc0 + ncols], in0=tmp[:, 0:ncols],
                                        in1=Rb[:, 0:ncols], op=AOP.mult)

        # ---------------- embed ----------------
        xt = big_p.tile([128, NKI, T], BF16, tag="big")
        nc.sync.dma_start(xt[:], d['xT'].rearrange("(k p) t -> p k t", p=128))
        for m in range(NDC):
            wtA = ws_p.tile([128, NDC, 128], BF16, tag="ws")
            wtB = ws_p.tile([128, NDC, 128], BF16, tag="ws")
            emb_ap = d['emb_w'][m].rearrange("p (k c) -> p k c", k=NKI)
            nc.sync.dma_start(wtA[:], emb_ap[:, 0:NDC, :])
            nc.sync.dma_start(wtB[:, 0:NKI - NDC, :], emb_ap[:, NDC:NKI, :])
            ps0 = ps_mm.tile([128, HT], F32, tag="mm")
            ps1 = ps_mm.tile([128, HT], F32, tag="mm")
            for k in range(NKI):
                wt = wtA[:, k, :] if k < NDC else wtB[:, k - NDC, :]
                nc.tensor.matmul(ps0[:], wt, xt[:, k, 0:HT],
                                 start=(k == 0), stop=(k == NKI - 1))
                nc.tensor.matmul(ps1[:], wt, xt[:, k, HT:T],
                                 start=(k == 0), stop=(k == NKI - 1))
            nc.vector.tensor_copy(hT[:, m, 0:HT], ps0[:])
            nc.vector.tensor_copy(hT[:, m, HT:T], ps1[:])

        # ---------------- layers ----------------
        for li in range(n_layers):
            last = (li == n_layers - 1) and (n_layers == L)
            bia = bias_p.tile([128, 64], F32, tag="bias")
            nc.sync.dma_start(bia[:], d['bias_all'][li])
            aT = aT_p.tile([128, NDC, T], BF16, tag="aT")
            ln_half(hT, 0, HT, aT)
            ln_half(hT, HT, HT, aT)
            # ---- Q, K (full T, weights loaded once) ----
            for mat, dst in ((0, qT), (1, kT)):
                for m in range(NDC):
                    wt = ws_p.tile([128, NDC, 128], BF16, tag="ws")
                    nc.sync.dma_start(wt[:],
                                      d['qkv_w'][li, mat, m].rearrange("p (k c) -> p k c", k=NDC))
                    ps0 = ps_mm.tile([128, HT], F32, tag="mm")
                    ps1 = ps_mm.tile([128, HT], F32, tag="mm")
                    for k in range(NDC):
                        nc.tensor.matmul(ps0[:], wt[:, k, :], aT[:, k, 0:HT],
                                         start=(k == 0), stop=(k == NDC - 1))
                        nc.tensor.matmul(ps1[:], wt[:, k, :], aT[:, k, HT:T],
                                         start=(k == 0), stop=(k == NDC - 1))
                    bcol = bia[:, mat * NDC + m:mat * NDC + m + 1]
                    nc.vector.tensor_scalar_add(dst[:, m, 0:HT], ps0[:], bcol)
                    nc.vector.tensor_scalar_add(dst[:, m, HT:T], ps1[:], bcol)
            # ---- V weights (resident for the layer) ----
            wv = wv_p.tile([128, NDC, D], BF16, tag="wv")
            nc.sync.dma_start(wv[:], d['wv_nat'][li].rearrange("k p n -> p k n"))
            for hf in range(2):
                hc0 = hf * HT
                # V (token-major, per batch)
                v = v_p.tile([64, HB, D], BF16, tag="v")
                for bi in range(HB):
                    bc0 = hc0 + bi * S
                    psv0 = ps_mm.tile([64, 512], F32, tag="mm")
                    psv1 = ps_mm.tile([64, 512], F32, tag="mm")
                    for k in range(NDC):
                        nc.tensor.matmul(psv0[0:S, :], aT[:, k, bc0:bc0 + S],
                                         wv[:, k, 0:512], start=(k == 0), stop=(k == NDC - 1))
                        nc.tensor.matmul(psv1[0:S, :], aT[:, k, bc0:bc0 + S],
                                         wv[:, k, 512:1024], start=(k == 0), stop=(k == NDC - 1))
                    nc.scalar.copy(v[0:S, bi, 0:512], psv0[0:S, :])
                    nc.scalar.copy(v[0:S, bi, 512:1024], psv1[0:S, :])
                # attention per batch
                for bi in range(HB):
                    bc0 = hc0 + bi * S
                    psE = ps_mm.tile([64, 8 * S], F32, tag="mm")
                    psO = ps_mm.tile([64, 8 * S], F32, tag="mm")
                    for c in range(NDC):
                        nc.tensor.matmul(psE[0:S, c * S:(c + 1) * S],
                                         kT[0:DK, c, bc0:bc0 + S], qT[0:DK, c, bc0:bc0 + S],
                                         start=True, stop=True)
                    for c in range(NDC):
                        nc.tensor.matmul(psO[0:S, c * S:(c + 1) * S],
                                         kT[DK:128, c, bc0:bc0 + S], qT[DK:128, c, bc0:bc0 + S],
                                         start=True, stop=True)
                    pTE = pt_p.tile([64, 8 * S], BF16, tag="pt")
                    pTO = pt_p.tile([64, 8 * S], BF16, tag="pt")
                    nc.scalar.activation(pTE[0:S, :], psE[0:S, :], EXP,
                                         bias=0.0, scale=1.0 / math.sqrt(DK))
                    nc.scalar.activation(pTO[0:S, :], psO[0:S, :], EXP,
                                         bias=0.0, scale=1.0 / math.sqrt(DK))
                    denE = ps_den.tile([64, 8 * S], F32, tag="den")
                    denO = ps_den.tile([64, 8 * S], F32, tag="den")
                    nc.tensor.matmul(denE[0:1, :], ones_cb[0:S, :], pTE[0:S, :],
                                     start=True, stop=True)
                    nc.tensor.matmul(denO[0:1, :], ones_cb[0:S, :], pTO[0:S, :],
                                     start=True, stop=True)
                    rd = rd_p.tile([1, 16 * S], F32R, tag="rd")
                    with nc.allow_low_precision(reason="softmax denom reciprocal"):
                        nc.vector.reciprocal(rd[:, 0:8 * S], denE[0:1, :])
                        nc.vector.reciprocal(rd[:, 8 * S:16 * S], denO[0:1, :])
                    rdB = ps_mm.tile([128, 8 * S], F32, tag="mm")
                    nc.tensor.matmul(rdB[:], sel[:, 0:128], rd[:, 0:8 * S],
                                     start=True, stop=False)
                    nc.tensor.matmul(rdB[:], sel[:, 128:256], rd[:, 8 * S:16 * S],
                                     start=False, stop=True)
                    rdS = sq_p.tile([128, 8 * S], F32R, tag="rds")
                    nc.scalar.copy(rdS[:], rdB[:])
                    po = ps_mm.tile([128, 8 * S], F32, tag="mm")
                    for c in range(NDC):
                        nc.tensor.matmul(po[0:DK, c * S:(c + 1) * S],
                                         v[0:S, bi, (2 * c) * DK:(2 * c + 1) * DK],
                                         pTE[0:S, c * S:(c + 1) * S], start=True, stop=True)
                    for c in range(NDC):
                        nc.tensor.matmul(po[DK:128, c * S:(c + 1) * S],
                                         v[0:S, bi, (2 * c + 1) * DK:(2 * c + 2) * DK],
                                         pTO[0:S, c * S:(c + 1) * S], start=True, stop=True)
                    nc.vector.tensor_tensor(
                        out=oT[:, :, bc0:bc0 + S],
                        in0=po[:].rearrange("p (c t) -> p c t", c=NDC),
                        in1=rdS[:].rearrange("p (c t) -> p c t", c=NDC),
                        op=AOP.mult)
            # ---- Wo + residual ----
            for m in range(NDC):
                wt = ws_p.tile([128, NDC, 128], BF16, tag="ws")
                nc.sync.dma_start(wt[:],
                                  d['wo_w'][li, m].rearrange("p (k c) -> p k c", k=NDC))
                ps0 = ps_mm.tile([128, HT], F32, tag="mm")
                ps1 = ps_mm.tile([128, HT], F32, tag="mm")
                for k in range(NDC):
                    nc.tensor.matmul(ps0[:], wt[:, k, :], oT[:, k, 0:HT],
                                     start=(k == 0), stop=(k == NDC - 1))
                    nc.tensor.matmul(ps1[:], wt[:, k, :], oT[:, k, HT:T],
                                     start=(k == 0), stop=(k == NDC - 1))
                bcol = bia[:, 16 + m:16 + m + 1]
                nc.vector.scalar_tensor_tensor(out=hT[:, m, 0:HT], in0=ps0[:], scalar=bcol,
                                               in1=hT[:, m, 0:HT], op0=AOP.add, op1=AOP.add)
                nc.vector.scalar_tensor_tensor(out=hT[:, m, HT:T], in0=ps1[:], scalar=bcol,
                                               in1=hT[:, m, HT:T], op0=AOP.add, op1=AOP.add)
            # ---- FFN ----
            if not last:
                aT2 = aT_p.tile([128, NDC, T], BF16, tag="aT")
                ln_half(hT, 0, HT, aT2)
                ln_half(hT, HT, HT, aT2)
                ffq = big_p.tile([128, NFC, T], BF16, tag="big")
                for m in range(NFC):
                    wt = ws_p.tile([128, NDC, 128], BF16, tag="ws")
                    nc.sync.dma_start(wt[:],
                                      d['w1_w'][li, m].rearrange("p (k c) -> p k c", k=NDC))
                    ps0 = ps_mm.tile([128, HT], F32, tag="mm")
                    ps1 = ps_mm.tile([128, HT], F32, tag="mm")
                    for k in range(NDC):
                        nc.tensor.matmul(ps0[:], wt[:, k, :], aT2[:, k, 0:HT],
                                         start=(k == 0), stop=(k == NDC - 1))
                        nc.tensor.matmul(ps1[:], wt[:, k, :], aT2[:, k, HT:T],
                                         start=(k == 0), stop=(k == NDC - 1))
                    bcol = bia[:, 24 + m:24 + m + 1]
                    nc.vector.tensor_scalar(out=ffq[:, m, 0:HT], in0=ps0[:], scalar1=bcol,
                                            scalar2=0.0, op0=AOP.add, op1=AOP.max)
                    nc.vector.tensor_scalar(out=ffq[:, m, HT:T], in0=ps1[:], scalar1=bcol,
                                            scalar2=0.0, op0=AOP.add, op1=AOP.max)
                for m in range(NDC):
                    w2ap = d['w2_w'][li, m].rearrange("p (k c) -> p k c", k=NFC)
                    w2ts = []
                    for kb in range(4):
                        w2t = ws_p.tile([128, NDC, 128], BF16, tag="ws")
                        nc.sync.dma_start(w2t[:], w2ap[:, kb * NDC:(kb + 1) * NDC, :])
                        w2ts.append(w2t)
                    ps0 = ps_mm.tile([128, HT], F32, tag="mm")
                    ps1 = ps_mm.tile([128, HT], F32, tag="mm")
                    for k in range(NFC):
                        w2t = w2ts[k // NDC]
                        nc.tensor.matmul(ps0[:], w2t[:, k % NDC, :], ffq[:, k, 0:HT],
                                         start=(k == 0), stop=(k == NFC - 1))
                        nc.tensor.matmul(ps1[:], w2t[:, k % NDC, :], ffq[:, k, HT:T],
                                         start=(k == 0), stop=(k == NFC - 1))
                    bcol = bia[:, 56 + m:56 + m + 1]
                    nc.vector.scalar_tensor_tensor(out=hT[:, m, 0:HT], in0=ps0[:], scalar=bcol,
                                                   in1=hT[:, m, 0:HT], op0=AOP.add, op1=AOP.add)
                    nc.vector.scalar_tensor_tensor(out=hT[:, m, HT:T], in0=ps1[:], scalar=bcol,
                                                   in1=hT[:, m, HT:T], op0=AOP.add, op1=AOP.add)
            else:
                # last layer: FFN only for the last token of each batch
                hL = sb1.tile([128, NDC, NB], F32R, tag="hL")
                for c in range(NDC):
                    nc.vector.tensor_copy(
                        hL[:, c, :],
                        hT[:, c, :].rearrange("p (b s) -> p b s", s=S)[:, :, S - 1])
                aL = sb1.tile([128, NDC, NB], BF16, tag="aL")
                ln_half(hL, 0, NB, aL)
                ffL = sb1.tile([128, NFC, NB], BF16, tag="ffL")
                for m in range(NFC):
                    wt = ws_p.tile([128, NDC, 128], BF16, tag="ws")
                    nc.sync.dma_start(wt[:],
                                      d['w1_w'][li, m].rearrange("p (k c) -> p k c", k=NDC))
                    ps = ps_mm.tile([128, HT], F32, tag="mm")
                    for k in range(NDC):
                        nc.tensor.matmul(ps[:, 0:NB], wt[:, k, :], aL[:, k, :],
                                         start=(k == 0), stop=(k == NDC - 1))
                    bcol = bia[:, 24 + m:24 + m + 1]
                    nc.vector.tensor_scalar(out=ffL[:, m, :], in0=ps[:, 0:NB], scalar1=bcol,
                                            scalar2=0.0, op0=AOP.add, op1=AOP.max)
                for m in range(NDC):
                    w2ap = d['w2_w'][li, m].rearrange("p (k c) -> p k c", k=NFC)
                    w2ts = []
                    for kb in range(4):
                        w2t = ws_p.tile([128, NDC, 128], BF16, tag="ws")
                        nc.sync.dma_start(w2t[:], w2ap[:, kb * NDC:(kb + 1) * NDC, :])
                        w2ts.append(w2t)
                    ps = ps_mm.tile([128, HT], F32, tag="mm")
                    for k in range(NFC):
                        nc.tensor.matmul(ps[:, 0:NB], w2ts[k // NDC][:, k % NDC, :], ffL[:, k, :],
                                         start=(k == 0), stop=(k == NFC - 1))
                    bcol = bia[:, 56 + m:56 + m + 1]
                    nc.vector.scalar_tensor_tensor(out=hL[:, m, :], in0=ps[:, 0:NB], scalar=bcol,
                                                   in1=hL[:, m, :], op0=AOP.add, op1=AOP.add)

        # ---------------- head ----------------
        if n_layers == L:
            src_pool = hL
        else:
            src_pool = sb1.tile([128, NDC, NB], F32R, tag="hL")
            for c in range(NDC):
                nc.vector.tensor_copy(
                    src_pool[:, c, :],
                    hT[:, c, :].rearrange("p (b s) -> p b s", s=S)[:, :, S - 1])
        pL = sb1.tile([128, NDC, NB], BF16, tag="pL")
        ln_half(src_pool, 0, NB, pL)
        cbT = bias_p.tile([128, 64], F32, tag="bias")
        nc.sync.dma_start(cbT[:, 0:NDC], d['cf_bT'][:])
        z1 = sb1.tile([128, NDC, NB], BF16, tag="z1")
        for m in range(NDC):
            wt = ws_p.tile([128, NDC, 128], BF16, tag="ws")
            nc.sync.dma_start(wt[:], d['cf_w'][m].rearrange("p (k c) -> p k c", k=NDC))
            ps = ps_mm.tile([128, HT], F32, tag="mm")
            for k in range(NDC):
                nc.tensor.matmul(ps[:, 0:NB], wt[:, k, :], pL[:, k, :],
                                 start=(k == 0), stop=(k == NDC - 1))
            nc.vector.tensor_scalar(out=z1[:, m, :], in0=ps[:, 0:NB], scalar1=cbT[:, m:m + 1],
                                    scalar2=0.0, op0=AOP.add, op1=AOP.max)
        fwt = sb1.tile([128, NDC, NCLS], BF16, tag="fwt")
        nc.sync.dma_start(fwt[:], d['fc_w'].rearrange("p (k c) -> p k c", k=NDC))
        fb = rows_p.tile([NCLS, 1], F32, tag="fb")
        nc.sync.dma_start(fb[:], d['fc_b'][:])
        ps = ps_mm.tile([128, HT], F32, tag="mm")
        for k in range(NDC):
            nc.tensor.matmul(ps[0:NCLS, 0:NB], fwt[:, k, :], z1[:, k, :],
                             start=(k == 0), stop=(k == NDC - 1))
        osb = sb1.tile([NCLS, NB], F32, tag="osb")
        nc.vector.tensor_scalar_add(osb[:], ps[0:NCLS, 0:NB], fb[:])
        nc.sync.dma_start(out[:], osb[:])


def _prep_weights(inputs, n_layers=L):
    import ml_dtypes
    f64 = np.float64
    bf16 = ml_dtypes.bfloat16

    def prep_lhsT(W):
        # W [K, M] -> [M/128, 128, (K/128)*128] : tile[p, k*128+c] = W[k*128+p, mb*128+c]
        K, M = W.shape
        nk, nm = K // 128, M // 128
        return np.ascontiguousarray(
            W.reshape(nk, 128, nm, 128).transpose(2, 1, 0, 3).reshape(nm, 128, nk * 128)
        ).astype(bf16)

    emb = inputs['embed_w'].astype(f64)          # [1200, 1024]
    pos = np.arange(S, dtype=f64)[:, None]
    div = np.exp(np.arange(0, D, 2, dtype=np.float32).astype(f64) * (-math.log(10000.0) / D))
    pe = np.zeros((S, D), f64)
    pe[:, 0::2] = np.sin(pos * div)
    pe[:, 1::2] = np.cos(pos * div)
    Wp = np.zeros((NIP, D), f64)
    Wp[:NI] = emb
    Wp[NI:NI + S] = pe
    g = {}
    g['emb_w'] = prep_lhsT(Wp)

    ln_g = inputs['ln_g'].astype(f64); ln_b = inputs['ln_b'].astype(f64)
    aw = inputs['attn_w'].astype(f64); ab = inputs['attn_b'].astype(f64)
    fw1 = inputs['ff_w1'].astype(f64); fb1 = inputs['ff_b1'].astype(f64)
    fw2 = inputs['ff_w2'].astype(f64); fb2 = inputs['ff_b2'].astype(f64)

    qkv_w = np.zeros((L, 2, NDC, 128, NDC * 128), bf16)
    wv_nat = np.zeros((L, NDC, 128, D), bf16)
    wo_w = np.zeros((L, NDC, 128, NDC * 128), bf16)
    w1_w = np.zeros((L, NFC, 128, NDC * 128), bf16)
    w2_w = np.zeros((L, NDC, 128, NFC * 128), bf16)
    bias_all = np.zeros((L, 128, 64), np.float32)

    for i in range(n_layers):
        g1, b1 = ln_g[i, 0][:, None], ln_b[i, 0]
        for mat in range(3):
            We = g1 * aw[i, mat]
            be = ab[i, mat] + b1 @ aw[i, mat]
            if mat == 2:
                wv_nat[i] = We.astype(bf16).reshape(NDC, 128, D)
                bv = be  # v bias folded into wo bias below (softmax rows sum to 1)
            else:
                qkv_w[i, mat] = prep_lhsT(We)
                bias_all[i, :, mat * NDC:(mat + 1) * NDC] = be.reshape(NDC, 128).T
        wo_w[i] = prep_lhsT(aw[i, 3])
        wo_be = ab[i, 3] + bv @ aw[i, 3]
        bias_all[i, :, 16:24] = wo_be.reshape(NDC, 128).T
        g2, b2 = ln_g[i, 1][:, None], ln_b[i, 1]
        W1e = g2 * fw1[i]
        b1e = fb1[i] + b2 @ fw1[i]
        w1_w[i] = prep_lhsT(W1e)
        bias_all[i, :, 24:56] = b1e.reshape(NFC, 128).T
        w2_w[i] = prep_lhsT(fw2[i])
        bias_all[i, :, 56:64] = fb2[i].reshape(NDC, 128).T

    g['qkv_w'] = qkv_w; g['wv_nat'] = wv_nat; g['wo_w'] = wo_w
    g['w1_w'] = w1_w; g['w2_w'] = w2_w; g['bias_all'] = bias_all

    inv = 1.0 / math.sqrt(1.0 + 1e-5)
    fin_g = inputs['fin_g'].astype(f64); fin_b = inputs['fin_b'].astype(f64)
    A1 = fin_g * inv * inputs['cf_bn_g'].astype(f64)
    C1 = fin_b * inv * inputs['cf_bn_g'].astype(f64) + inputs['cf_bn_b'].astype(f64)
    cfw = inputs['cf_w'].astype(f64)
    cf_we = A1[:, None] * cfw
    cf_be = inputs['cf_b'].astype(f64) + C1 @ cfw
    g['cf_w'] = prep_lhsT(cf_we)
    g['cf_bT'] = cf_be.reshape(NDC, 128).T.astype(np.float32)
    A2 = inv * inputs['fc_bn_g'].astype(f64)
    C2 = inputs['fc_bn_b'].astype(f64)
    fcw = inputs['fc_w'].astype(f64)
    fc_we = A2[:, None] * fcw
    fc_be = inputs['fc_b'].astype(f64) + C2 @ fcw
    g['fc_w'] = np.ascontiguousarray(
        fc_we.reshape(NDC, 128, NCLS).transpose(1, 0, 2).reshape(128, NDC * NCLS)
    ).astype(bf16)
    g['fc_b'] = fc_be.reshape(NCLS, 1).astype(np.float32)
    g['ones'] = np.ones((128, 512), np.float32)
    selm = np.zeros((1, 256), np.float32)
    selm[0, 0:64] = 1.0
    selm[0, 192:256] = 1.0
    g['sel'] = selm
    return g


def _run_timed(nc, in_maps, n_iters=10):
    """Mirror bass2jax.run_bass_via_pjrt (no donation), time steady-state execs."""
    import time
    import jax
    import numpy as _np
    from jax.experimental.shard_map import shard_map
    from jax.sharding import Mesh, PartitionSpec, NamedSharding
    from concourse import bass2jax as b2j
    from concourse import mybir as _mb

    b2j.install_neuronx_cc_hook()
    n_cores = len(in_maps)
    partition_name = nc.partition_id_tensor.name if nc.partition_id_tensor else None
    in_names, out_names, out_avals, zero_outs = [], [], [], []
    for alloc in nc.m.functions[0].allocations:
        if not isinstance(alloc, _mb.MemoryLocationSet):
            continue
        name = alloc.memorylocations[0].name
        if alloc.kind == "ExternalInput":
            if name != partition_name:
                in_names.append(name)
        elif alloc.kind == "ExternalOutput":
            shape = tuple(alloc.tensor_shape)
            dtype = _mb.dt.np(alloc.dtype)
            out_names.append(name)
            out_avals.append(jax.core.ShapedArray(shape, dtype))
            zero_outs.append(_np.zeros(shape, dtype))
    n_params = len(in_names)
    all_in_names = list(in_names) + list(out_names)
    if partition_name is not None:
        all_in_names.append(partition_name)

    def _body(*args):
        operands = list(args)
        if partition_name is not None:
            operands.append(b2j.partition_id_tensor())
        outs = b2j._bass_exec_p.bind(
            *operands,
            out_avals=tuple(out_avals),
            in_names=tuple(all_in_names),
            out_names=tuple(out_names),
            lowering_input_output_aliases=(),
            sim_require_finite=True,
            sim_require_nnan=True,
            nc=nc,
        )
        return tuple(outs)

    devices = jax.devices()[:n_cores]
    mesh = Mesh(_np.asarray(devices), ("core",))
    spec = PartitionSpec("core")
    sharded = jax.jit(shard_map(
        _body, mesh=mesh, in_specs=(spec,) * (n_params + len(out_names)),
        out_specs=(spec,) * len(out_names), check_rep=False))
    sh = NamedSharding(mesh, spec)
    concat_in = [
        jax.device_put(_np.concatenate([_np.asarray(m[name]) for m in in_maps], axis=0), sh)
        for name in in_names
    ]
    concat_zeros = [
        jax.device_put(_np.zeros((n_cores * z.shape[0], *z.shape[1:]), z.dtype), sh)
        for z in zero_outs
    ]
    # Warm up until per-call time stabilizes (NEFF/IRAM load, DMA ring setup,
    # transport ramp all land in the first executions), then time steady state.
    prev = None
    for _ in range(6):
        tw0 = time.time()
        for _ in range(5):
            outs = sharded(*concat_in, *concat_zeros)
        jax.block_until_ready(outs)
        tw1 = time.time()
        cur = (tw1 - tw0) / 5
        if prev is not None and cur >= prev * 0.9:
            break
        prev = cur
    t0 = time.time()
    for _ in range(n_iters):
        outs = sharded(*concat_in, *concat_zeros)
    jax.block_until_ready(outs)
    t1 = time.time()
    per_call_ns = (t1 - t0) / n_iters * 1e9
    results = [
        {name: _np.asarray(outs[i]).reshape(n_cores, *out_avals[i].shape)[c]
         for i, name in enumerate(out_names)}
        for c in range(n_cores)
    ]
    return results, per_call_ns


def kernel(**inputs):
    global LAST_EXEC_NS
    import ml_dtypes
    bf16 = ml_dtypes.bfloat16
    n_layers = int(inputs.pop('_n_layers', L))
    if n_layers not in _CACHE:
        _CACHE[n_layers] = _build(n_layers)
    nc = _CACHE[n_layers]
    g = _prep_weights(inputs, n_layers)

    x = inputs['x']
    xr = x.reshape(B, S, NI)
    in_maps = []
    for ci in range(NCORES):
        xc = xr[ci * NB:(ci + 1) * NB].astype(np.float32)  # [16, 50, 1200]
        xa = np.zeros((NB, S, NIP), np.float32)
        xa[:, :, :NI] = xc
        xa[np.arange(NB)[:, None], np.arange(S)[None, :], NI + np.arange(S)[None, :]] = 1.0
        # xT [NIP, T]: feature-major, tokens (b, s)
        xT = np.ascontiguousarray(xa.reshape(T, NIP).T).astype(bf16)
        m = dict(g)
        m['xT'] = xT
        in_maps.append(m)

    if TRACE:
        results, per_call_ns = _run_timed(nc, in_maps)
        LAST_EXEC_NS = int(per_call_ns)
    else:
        res = run_bass_kernel_spmd(nc, in_maps, core_ids=list(range(NCORES)))
        LAST_EXEC_NS = res.exec_time_ns
        results = res.results
    outs = [r['out'].T for r in results]   # each [NB, NCLS]
    return np.concatenate(outs, axis=0).astype(np.float32)


# revision 13
# speedup vs baseline: 4.6895x; 4.1694x over previous
import sys
sys.path.insert(0, '/opt/trn_rl_repo')
import numpy as np
import math

import concourse.bass as bass
import concourse.mybir as mybir
import concourse.tile as tile
from concourse import bacc
from concourse.bass_utils import run_bass_kernel_spmd

# Problem dims
B, SL, CH, HZ = 128, 5000, 12, 100
L, D, DFF, H, NCLS = 5, 1024, 4096, 16, 71
NI = CH * HZ          # 1200
S = SL // HZ          # 50
NCORES = 8
NB = B // NCORES      # 16 batches per core
T = NB * S            # 800 tokens per core
NIP = 1280            # padded input-feature dim (1200 + 50 one-hot pos rows)
NKI = NIP // 128      # 10 input k-chunks
DK = D // H           # 64
NDC = D // 128        # 8 d-chunks
NFC = DFF // 128      # 32 ff-chunks
HB = NB // 2          # 8 batches per half
HT = HB * S           # 400 tokens per half

F32R = mybir.dt.float32r
F32 = mybir.dt.float32
BF16 = mybir.dt.bfloat16
EXP = mybir.ActivationFunctionType.Exp
RELU = mybir.ActivationFunctionType.Relu
SQRT = mybir.ActivationFunctionType.Sqrt
AOP = mybir.AluOpType

TRACE = False
LAST_EXEC_NS = None
_CACHE = {}


def _build(n_layers=L):
    nc = bacc.Bacc(None)
    d = {}
    d['xT'] = nc.dram_tensor("xT", [NIP, T], BF16, kind="ExternalInput")
    d['ones'] = nc.dram_tensor("ones", [128, 512], F32R, kind="ExternalInput")
    d['sel'] = nc.dram_tensor("sel", [1, 256], F32R, kind="ExternalInput")
    d['emb_w'] = nc.dram_tensor("emb_w", [NDC, 128, NKI * 128], BF16, kind="ExternalInput")
    d['qkv_w'] = nc.dram_tensor("qkv_w", [L, 2, NDC, 128, NDC * 128], BF16, kind="ExternalInput")
    d['wv_nat'] = nc.dram_tensor("wv_nat", [L, NDC, 128, D], BF16, kind="ExternalInput")
    d['wo_w'] = nc.dram_tensor("wo_w", [L, NDC, 128, NDC * 128], BF16, kind="ExternalInput")
    d['w1_w'] = nc.dram_tensor("w1_w", [L, NFC, 128, NDC * 128], BF16, kind="ExternalInput")
    d['w2_w'] = nc.dram_tensor("w2_w", [L, NDC, 128, NFC * 128], BF16, kind="ExternalInput")
    # bias_all cols: 0:8 q, 8:16 k, 16:24 wo, 24:56 w1, 56:64 w2
    d['bias_all'] = nc.dram_tensor("bias_all", [L, 128, 64], F32, kind="ExternalInput")
    d['cf_w'] = nc.dram_tensor("cf_w", [NDC, 128, NDC * 128], BF16, kind="ExternalInput")
    d['cf_bT'] = nc.dram_tensor("cf_bT", [128, NDC], F32, kind="ExternalInput")
    d['fc_w'] = nc.dram_tensor("fc_w", [128, NDC * NCLS], BF16, kind="ExternalInput")
    d['fc_b'] = nc.dram_tensor("fc_b", [NCLS, 1], F32, kind="ExternalInput")
    out = nc.dram_tensor("out", [NCLS, NB], F32, kind="ExternalOutput")

    with tile.TileContext(nc) as tc:
        _emit(nc, tc, d, out, n_layers)
    nc.compile()
    return nc


def _emit(nc, tc, d, out, n_layers):
    import contextlib
    ctx = contextlib.ExitStack()
    with ctx:
        sb1 = ctx.enter_context(tc.tile_pool(name="sb1", bufs=1))
        aT_p = ctx.enter_context(tc.tile_pool(name="aTp", bufs=1))
        big_p = ctx.enter_context(tc.tile_pool(name="bigp", bufs=1))
        sq_p = ctx.enter_context(tc.tile_pool(name="sqp", bufs=2))
        ws_p = ctx.enter_context(tc.tile_pool(name="wsp", bufs=6))
        wv_p = ctx.enter_context(tc.tile_pool(name="wvp", bufs=1))
        v_p = ctx.enter_context(tc.tile_pool(name="vp", bufs=1))
        pt_p = ctx.enter_context(tc.tile_pool(name="ptp", bufs=4))
        rd_p = ctx.enter_context(tc.tile_pool(name="rdp", bufs=2))
        rows_p = ctx.enter_context(tc.tile_pool(name="rowsp", bufs=4))
        bias_p = ctx.enter_context(tc.tile_pool(name="biasp", bufs=2))
        ps_mm = ctx.enter_context(tc.tile_pool(name="psmm", bufs=6, space="PSUM"))
        ps_den = ctx.enter_context(tc.tile_pool(name="psden", bufs=2, space="PSUM"))

        # persistent tiles
        hT = sb1.tile([128, NDC, T], F32R, tag="hT")
        qT = sb1.tile([128, NDC, T], BF16, tag="qT")
        kT = sb1.tile([128, NDC, T], BF16, tag="kT")
        oT = sb1.tile([128, NDC, T], BF16, tag="oT")
        ones_c = sb1.tile([128, 1], F32R, tag="ones_c")
        ones_cb = sb1.tile([128, 1], BF16, tag="ones_cb")
        ones_r = sb1.tile([1, 512], F32R, tag="ones_r")
        nc.sync.dma_start(ones_c[:], d['ones'][:, 0:1])
        nc.sync.dma_start(ones_r[:], d['ones'][0:1, :])
        nc.vector.tensor_copy(ones_cb[:], ones_c[:])
        # sel [1,256]: cols 0:128 = row-half-E selector (1s at 0:64),
        # cols 128:256 = row-half-O selector (1s at 64:128)
        sel = sb1.tile([1, 256], F32R, tag="sel")
        nc.sync.dma_start(sel[:], d['sel'][:])

        def ln_half(src, hc0, ncols, dst):
            """dst[:, c, hc0:hc0+ncols](bf16) = LN over feature dim of
            src[:, c, hc0:hc0+ncols] (fp32r, feature-major)."""
            Dn = float(NDC * 128)
            cm = 1.0 / Dn
            cv2 = 1.0 / (Dn - 1.0)
            cv1 = -1.0 / (Dn * (Dn - 1.0))
            s1 = ps_den.tile([64, ncols], F32, tag="den")
            s2 = ps_den.tile([64, ncols], F32, tag="den")
            for c in range(NDC):
                sq = sq_p.tile([128, HT], F32R, tag="sq")
                nc.vector.tensor_tensor(out=sq[:, 0:ncols], in0=src[:, c, hc0:hc0 + ncols],
                                        in1=src[:, c, hc0:hc0 + ncols], op=AOP.mult)
                nc.tensor.matmul(s1[0:1, :], ones_c[:], src[:, c, hc0:hc0 + ncols],
                                 start=(c == 0), stop=(c == NDC - 1))
                nc.tensor.matmul(s2[0:1, :], ones_c[:], sq[:, 0:ncols],
                                 start=(c == 0), stop=(c == NDC - 1))
            m_row = rows_p.tile([1, HT], F32R, tag="rows")
            t1 = rows_p.tile([1, HT], F32, tag="rows")
            tv = rows_p.tile([1, HT], F32, tag="rows")
            r_row = rows_p.tile([1, HT], F32R, tag="rows")
            nc.vector.tensor_scalar_mul(m_row[:, 0:ncols], s1[0:1, :], cm)
            # cv1*s1^2 == (cv1*Dn*Dn) * m_row^2  (m_row is SBUF; s1 is PSUM)
            nc.vector.scalar_tensor_tensor(out=t1[:, 0:ncols], in0=m_row[:, 0:ncols],
                                           scalar=cv1 * Dn * Dn, in1=m_row[:, 0:ncols],
                                           op0=mybir.AluOpType.mult,
                                           op1=mybir.AluOpType.mult)
            nc.vector.scalar_tensor_tensor(out=tv[:, 0:ncols], in0=s2[0:1, :], scalar=cv2,
                                           in1=t1[:, 0:ncols], op0=mybir.AluOpType.mult,
                                           op1=mybir.AluOpType.add)
            nc.scalar.activation(tv[:, 0:ncols], tv[:, 0:ncols], SQRT, bias=0.0, scale=1.0)
            nc.vector.tensor_scalar_add(tv[:, 0:ncols], tv[:, 0:ncols], 1e-6)
            with nc.allow_low_precision(reason="fp32r rounding of 1/(std+eps)"):
                nc.vector.reciprocal(r_row[:, 0:ncols], tv[:, 0:ncols])
            Mb = ps_mm.tile([128, HT], F32, tag="mm")
            Rb = ps_mm.tile([128, HT], F32, tag="mm")
            nc.tensor.matmul(Mb[:, 0:ncols], ones_r[0:1, 0:128], m_row[:, 0:ncols],
                             start=True, stop=True)
            nc.tensor.matmul(Rb[:, 0:ncols], ones_r[0:1, 0:128], r_row[:, 0:ncols],
                             start=True, stop=True)
            for c in range(NDC):
                tmp = sq_p.tile([128, HT], F32R, tag="sq")
                nc.vector.tensor_tensor(out=tmp[:, 0:ncols], in0=src[:, c, hc0:hc0 + ncols],
                                        in1=Mb[:, 0:ncols], op=AOP.subtract)
                nc.vector.tensor_tensor(out=dst[:, c, hc0:hc0 + ncols], in0=tmp[:, 0:ncols],
                                        in1=Rb[:, 0:ncols], op=AOP.mult)

        # ---------------- embed ----------------
        xt = big_p.tile([128, NKI, T], BF16, tag="big")
        nc.sync.dma_start(xt[:], d['xT'].rearrange("(k p) t -> p k t", p=128))
        for m in range(NDC):
            wtA = ws_p.tile([128, NDC, 128], BF16, tag="ws")
            wtB = ws_p.tile([128, NDC, 128], BF16, tag="ws")
            emb_ap = d['emb_w'][m].rearrange("p (k c) -> p k c", k=NKI)
            nc.sync.dma_start(wtA[:], emb_ap[:, 0:NDC, :])
            nc.sync.dma_start(wtB[:, 0:NKI - NDC, :], emb_ap[:, NDC:NKI, :])
            ps0 = ps_mm.tile([128, HT], F32, tag="mm")
            ps1 = ps_mm.tile([128, HT], F32, tag="mm")
            for k in range(NKI):
                wt = wtA[:, k, :] if k < NDC else wtB[:, k - NDC, :]
                nc.tensor.matmul(ps0[:], wt, xt[:, k, 0:HT],
                                 start=(k == 0), stop=(k == NKI - 1))
                nc.tensor.matmul(ps1[:], wt, xt[:, k, HT:T],
                                 start=(k == 0), stop=(k == NKI - 1))
            nc.vector.tensor_copy(hT[:, m, 0:HT], ps0[:])
            nc.vector.tensor_copy(hT[:, m, HT:T], ps1[:])

        # ---------------- layers ----------------
        for li in range(n_layers):
            last = (li == n_layers - 1) and (n_layers == L)
            bia = bias_p.tile([128, 64], F32, tag="bias")
            nc.sync.dma_start(bia[:], d['bias_all'][li])
            aT = aT_p.tile([128, NDC, T], BF16, tag="aT")
            ln_half(hT, 0, HT, aT)
            ln_half(hT, HT, HT, aT)
            # ---- Q, K (full T, weights loaded once) ----
            for mat, dst in ((0, qT), (1, kT)):
                for m in range(NDC):
                    wt = ws_p.tile([128, NDC, 128], BF16, tag="ws")
                    nc.sync.dma_start(wt[:],
                                      d['qkv_w'][li, mat, m].rearrange("p (k c) -> p k c", k=NDC))
                    ps0 = ps_mm.tile([128, HT], F32, tag="mm")
                    ps1 = ps_mm.tile([128, HT], F32, tag="mm")
                    for k in range(NDC):
                        nc.tensor.matmul(ps0[:], wt[:, k, :], aT[:, k, 0:HT],
                                         start=(k == 0), stop=(k == NDC - 1))
                        nc.tensor.matmul(ps1[:], wt[:, k, :], aT[:, k, HT:T],
                                         start=(k == 0), stop=(k == NDC - 1))
                    bcol = bia[:, mat * NDC + m:mat * NDC + m + 1]
                    nc.vector.tensor_scalar_add(dst[:, m, 0:HT], ps0[:], bcol)
                    nc.vector.tensor_scalar_add(dst[:, m, HT:T], ps1[:], bcol)
            # ---- V weights (resident for the layer) ----
            wv = wv_p.tile([128, NDC, D], BF16, tag="wv")
            nc.sync.dma_start(wv[:], d['wv_nat'][li].rearrange("k p n -> p k n"))
            for hf in range(2):
                hc0 = hf * HT
                # V (token-major, per batch)
                v = v_p.tile([64, HB, D], BF16, tag="v")
                for bi in range(HB):
                    bc0 = hc0 + bi * S
                    psv0 = ps_mm.tile([64, 512], F32, tag="mm")
                    psv1 = ps_mm.tile([64, 512], F32, tag="mm")
                    for k in range(NDC):
                        nc.tensor.matmul(psv0[0:S, :], aT[:, k, bc0:bc0 + S],
                                         wv[:, k, 0:512], start=(k == 0), stop=(k == NDC - 1))
                        nc.tensor.matmul(psv1[0:S, :], aT[:, k, bc0:bc0 + S],
                                         wv[:, k, 512:1024], start=(k == 0), stop=(k == NDC - 1))
                    nc.scalar.copy(v[0:S, bi, 0:512], psv0[0:S, :])
                    nc.scalar.copy(v[0:S, bi, 512:1024], psv1[0:S, :])
                # attention per batch
                for bi in range(HB):
                    bc0 = hc0 + bi * S
                    psE = ps_mm.tile([64, 8 * S], F32, tag="mm")
                    psO = ps_mm.tile([64, 8 * S], F32, tag="mm")
                    for c in range(NDC):
                        nc.tensor.matmul(psE[0:S, c * S:(c + 1) * S],
                                         kT[0:DK, c, bc0:bc0 + S], qT[0:DK, c, bc0:bc0 + S],
                                         start=True, stop=True)
                    for c in range(NDC):
                        nc.tensor.matmul(psO[0:S, c * S:(c + 1) * S],
                                         kT[DK:128, c, bc0:bc0 + S], qT[DK:128, c, bc0:bc0 + S],
                                         start=True, stop=True)
                    pTE = pt_p.tile([64, 8 * S], BF16, tag="pt")
                    pTO = pt_p.tile([64, 8 * S], BF16, tag="pt")
                    nc.scalar.activation(pTE[0:S, :], psE[0:S, :], EXP,
                                         bias=0.0, scale=1.0 / math.sqrt(DK))
                    nc.scalar.activation(pTO[0:S, :], psO[0:S, :], EXP,
                                         bias=0.0, scale=1.0 / math.sqrt(DK))
                    denE = ps_den.tile([64, 8 * S], F32, tag="den")
                    denO = ps_den.tile([64, 8 * S], F32, tag="den")
                    nc.tensor.matmul(denE[0:1, :], ones_cb[0:S, :], pTE[0:S, :],
                                     start=True, stop=True)
                    nc.tensor.matmul(denO[0:1, :], ones_cb[0:S, :], pTO[0:S, :],
                                     start=True, stop=True)
                    rd = rd_p.tile([1, 16 * S], F32R, tag="rd")
                    with nc.allow_low_precision(reason="softmax denom reciprocal"):
                        nc.vector.reciprocal(rd[:, 0:8 * S], denE[0:1, :])
                        nc.vector.reciprocal(rd[:, 8 * S:16 * S], denO[0:1, :])
                    rdB = ps_mm.tile([128, 8 * S], F32, tag="mm")
                    nc.tensor.matmul(rdB[:], sel[:, 0:128], rd[:, 0:8 * S],
                                     start=True, stop=False)
                    nc.tensor.matmul(rdB[:], sel[:, 128:256], rd[:, 8 * S:16 * S],
                                     start=False, stop=True)
                    rdS = sq_p.tile([128, 8 * S], F32R, tag="rds")
                    nc.scalar.copy(rdS[:], rdB[:])
                    po = ps_mm.tile([128, 8 * S], F32, tag="mm")
                    for c in range(NDC):
                        nc.tensor.matmul(po[0:DK, c * S:(c + 1) * S],
                                         v[0:S, bi, (2 * c) * DK:(2 * c + 1) * DK],
                                         pTE[0:S, c * S:(c + 1) * S], start=True, stop=True)
                    for c in range(NDC):
                        nc.tensor.matmul(po[DK:128, c * S:(c + 1) * S],
                                         v[0:S, bi, (2 * c + 1) * DK:(2 * c + 2) * DK],
                                         pTO[0:S, c * S:(c + 1) * S], start=True, stop=True)
                    nc.vector.tensor_tensor(
                        out=oT[:, :, bc0:bc0 + S],
                        in0=po[:].rearrange("p (c t) -> p c t", c=NDC),
                        in1=rdS[:].rearrange("p (c t) -> p c t", c=NDC),
                        op=AOP.mult)
            # ---- Wo + residual ----
            for m in range(NDC):
                wt = ws_p.tile([128, NDC, 128], BF16, tag="ws")
                nc.sync.dma_start(wt[:],
                                  d['wo_w'][li, m].rearrange("p (k c) -> p k c", k=NDC))
                ps0 = ps_mm.tile([128, HT], F32, tag="mm")
                ps1 = ps_mm.tile([128, HT], F32, tag="mm")
                for k in range(NDC):
                    nc.tensor.matmul(ps0[:], wt[:, k, :], oT[:, k, 0:HT],
                                     start=(k == 0), stop=(k == NDC - 1))
                    nc.tensor.matmul(ps1[:], wt[:, k, :], oT[:, k, HT:T],
                                     start=(k == 0), stop=(k == NDC - 1))
                bcol = bia[:, 16 + m:16 + m + 1]
                nc.vector.scalar_tensor_tensor(out=hT[:, m, 0:HT], in0=ps0[:], scalar=bcol,
                                               in1=hT[:, m, 0:HT], op0=AOP.add, op1=AOP.add)
                nc.vector.scalar_tensor_tensor(out=hT[:, m, HT:T], in0=ps1[:], scalar=bcol,
                                               in1=hT[:, m, HT:T], op0=AOP.add, op1=AOP.add)
            # ---- FFN ----
            if not last:
                aT2 = aT_p.tile([128, NDC, T], BF16, tag="aT")
                ln_half(hT, 0, HT, aT2)
                ln_half(hT, HT, HT, aT2)
                ffq = big_p.tile([128, NFC, T], BF16, tag="big")
                for m in range(NFC):
                    wt = ws_p.tile([128, NDC, 128], BF16, tag="ws")
                    nc.sync.dma_start(wt[:],
                                      d['w1_w'][li, m].rearrange("p (k c) -> p k c", k=NDC))
                    ps0 = ps_mm.tile([128, HT], F32, tag="mm")
                    ps1 = ps_mm.tile([128, HT], F32, tag="mm")
                    for k in range(NDC):
                        nc.tensor.matmul(ps0[:], wt[:, k, :], aT2[:, k, 0:HT],
                                         start=(k == 0), stop=(k == NDC - 1))
                        nc.tensor.matmul(ps1[:], wt[:, k, :], aT2[:, k, HT:T],
                                         start=(k == 0), stop=(k == NDC - 1))
                    bcol = bia[:, 24 + m:24 + m + 1]
                    nc.vector.tensor_scalar(out=ffq[:, m, 0:HT], in0=ps0[:], scalar1=bcol,
                                            scalar2=0.0, op0=AOP.add, op1=AOP.max)
                    nc.vector.tensor_scalar(out=ffq[:, m, HT:T], in0=ps1[:], scalar1=bcol,
                                            scalar2=0.0, op0=AOP.add, op1=AOP.max)
                for m in range(NDC):
                    w2ap = d['w2_w'][li, m].rearrange("p (k c) -> p k c", k=NFC)
                    w2ts = []
                    for kb in range(4):
                        w2t = ws_p.tile([128, NDC, 128], BF16, tag="ws")
                        nc.sync.dma_start(w2t[:], w2ap[:, kb * NDC:(kb + 1) * NDC, :])
                        w2ts.append(w2t)
                    ps0 = ps_mm.tile([128, HT], F32, tag="mm")
                    ps1 = ps_mm.tile([128, HT], F32, tag="mm")
                    for k in range(NFC):
                        w2t = w2ts[k // NDC]
                        nc.tensor.matmul(ps0[:], w2t[:, k % NDC, :], ffq[:, k, 0:HT],
                                         start=(k == 0), stop=(k == NFC - 1))
                        nc.tensor.matmul(ps1[:], w2t[:, k % NDC, :], ffq[:, k, HT:T],
                                         start=(k == 0), stop=(k == NFC - 1))
                    bcol = bia[:, 56 + m:56 + m + 1]
                    nc.vector.scalar_tensor_tensor(out=hT[:, m, 0:HT], in0=ps0[:], scalar=bcol,
                                                   in1=hT[:, m, 0:HT], op0=AOP.add, op1=AOP.add)
                    nc.vector.scalar_tensor_tensor(out=hT[:, m, HT:T], in0=ps1[:], scalar=bcol,
                                                   in1=hT[:, m, HT:T], op0=AOP.add, op1=AOP.add)
            else:
                # last layer: FFN only for the last token of each batch
                hL = sb1.tile([128, NDC, NB], F32R, tag="hL")
                for c in range(NDC):
                    nc.vector.tensor_copy(
                        hL[:, c, :],
                        hT[:, c, :].rearrange("p (b s) -> p b s", s=S)[:, :, S - 1])
                aL = sb1.tile([128, NDC, NB], BF16, tag="aL")
                ln_half(hL, 0, NB, aL)
                ffL = sb1.tile([128, NFC, NB], BF16, tag="ffL")
                for m in range(NFC):
                    wt = ws_p.tile([128, NDC, 128], BF16, tag="ws")
                    nc.sync.dma_start(wt[:],
                                      d['w1_w'][li, m].rearrange("p (k c) -> p k c", k=NDC))
                    ps = ps_mm.tile([128, HT], F32, tag="mm")
                    for k in range(NDC):
                        nc.tensor.matmul(ps[:, 0:NB], wt[:, k, :], aL[:, k, :],
                                         start=(k == 0), stop=(k == NDC - 1))
                    bcol = bia[:, 24 + m:24 + m + 1]
                    nc.vector.tensor_scalar(out=ffL[:, m, :], in0=ps[:, 0:NB], scalar1=bcol,
                                            scalar2=0.0, op0=AOP.add, op1=AOP.max)
                for m in range(NDC):
                    w2ap = d['w2_w'][li, m].rearrange("p (k c) -> p k c", k=NFC)
                    w2ts = []
                    for kb in range(4):
                        w2t = ws_p.tile([128, NDC, 128], BF16, tag="ws")
                        nc.sync.dma_start(w2t[:], w2ap[:, kb * NDC:(kb + 1) * NDC, :])
                        w2ts.append(w2t)
                    ps = ps_mm.tile([128, HT], F32, tag="mm")
                    for k in range(NFC):
                        nc.tensor.matmul(ps[:, 0:NB], w2ts[k // NDC][:, k % NDC, :], ffL[:, k, :],
                                         start=(k == 0), stop=(k == NFC - 1))
                    bcol = bia[:, 56 + m:56 + m + 1]
                    nc.vector.scalar_tensor_tensor(out=hL[:, m, :], in0=ps[:, 0:NB], scalar=bcol,
                                                   in1=hL[:, m, :], op0=AOP.add, op1=AOP.add)

        # ---------------- head ----------------
        if n_layers == L:
            src_pool = hL
        else:
            src_pool = sb1.tile([128, NDC, NB], F32R, tag="hL")
            for c in range(NDC):
                nc.vector.tensor_copy(
                    src_pool[:, c, :],
                    hT[:, c, :].rearrange("p (b s) -> p b s", s=S)[:, :, S - 1])
        pL = sb1.tile([128, NDC, NB], BF16, tag="pL")
        ln_half(src_pool, 0, NB, pL)
        cbT = bias_p.tile([128, 64], F32, tag="bias")
        nc.sync.dma_start(cbT[:, 0:NDC], d['cf_bT'][:])
        z1 = sb1.tile([128, NDC, NB], BF16, tag="z1")
        for m in range(NDC):
            wt = ws_p.tile([128, NDC, 128], BF16, tag="ws")
            nc.sync.dma_start(wt[:], d['cf_w'][m].rearrange("p (k c) -> p k c", k=NDC))
            ps = ps_mm.tile([128, HT], F32, tag="mm")
            for k in range(NDC):
                nc.tensor.matmul(ps[:, 0:NB], wt[:, k, :], pL[:, k, :],
                                 start=(k == 0), stop=(k == NDC - 1))
            nc.vector.tensor_scalar(out=z1[:, m, :], in0=ps[:, 0:NB], scalar1=cbT[:, m:m + 1],
                                    scalar2=0.0, op0=AOP.add, op1=AOP.max)
        fwt = sb1.tile([128, NDC, NCLS], BF16, tag="fwt")
        nc.sync.dma_start(fwt[:], d['fc_w'].rearrange("p (k c) -> p k c", k=NDC))
        fb = rows_p.tile([NCLS, 1], F32, tag="fb")
        nc.sync.dma_start(fb[:], d['fc_b'][:])
        ps = ps_mm.tile([128, HT], F32, tag="mm")
        for k in range(NDC):
            nc.tensor.matmul(ps[0:NCLS, 0:NB], fwt[:, k, :], z1[:, k, :],
                             start=(k == 0), stop=(k == NDC - 1))
        osb = sb1.tile([NCLS, NB], F32, tag="osb")
        nc.vector.tensor_scalar_add(osb[:], ps[0:NCLS, 0:NB], fb[:])
        nc.sync.dma_start(out[:], osb[:])


def _prep_weights(inputs, n_layers=L):
    import ml_dtypes
    f64 = np.float64
    bf16 = ml_dtypes.bfloat16

    def prep_lhsT(W):
        # W [K, M] -> [M/128, 128, (K/128)*128] : tile[p, k*128+c] = W[k*128+p, mb*128+c]
        K, M = W.shape
        nk, nm = K // 128, M // 128
        return np.ascontiguousarray(
            W.reshape(nk, 128, nm, 128).transpose(2, 1, 0, 3).reshape(nm, 128, nk * 128)
        ).astype(bf16)

    emb = inputs['embed_w'].astype(f64)          # [1200, 1024]
    pos = np.arange(S, dtype=f64)[:, None]
    div = np.exp(np.arange(0, D, 2, dtype=np.float32).astype(f64) * (-math.log(10000.0) / D))
    pe = np.zeros((S, D), f64)
    pe[:, 0::2] = np.sin(pos * div)
    pe[:, 1::2] = np.cos(pos * div)
    Wp = np.zeros((NIP, D), f64)
    Wp[:NI] = emb
    Wp[NI:NI + S] = pe
    g = {}
    g['emb_w'] = prep_lhsT(Wp)

    ln_g = inputs['ln_g'].astype(f64); ln_b = inputs['ln_b'].astype(f64)
    aw = inputs['attn_w'].astype(f64); ab = inputs['attn_b'].astype(f64)
    fw1 = inputs['ff_w1'].astype(f64); fb1 = inputs['ff_b1'].astype(f64)
    fw2 = inputs['ff_w2'].astype(f64); fb2 = inputs['ff_b2'].astype(f64)

    qkv_w = np.zeros((L, 2, NDC, 128, NDC * 128), bf16)
    wv_nat = np.zeros((L, NDC, 128, D), bf16)
    wo_w = np.zeros((L, NDC, 128, NDC * 128), bf16)
    w1_w = np.zeros((L, NFC, 128, NDC * 128), bf16)
    w2_w = np.zeros((L, NDC, 128, NFC * 128), bf16)
    bias_all = np.zeros((L, 128, 64), np.float32)

    for i in range(n_layers):
        g1, b1 = ln_g[i, 0][:, None], ln_b[i, 0]
        for mat in range(3):
            We = g1 * aw[i, mat]
            be = ab[i, mat] + b1 @ aw[i, mat]
            if mat == 2:
                wv_nat[i] = We.astype(bf16).reshape(NDC, 128, D)
                bv = be  # v bias folded into wo bias below (softmax rows sum to 1)
            else:
                qkv_w[i, mat] = prep_lhsT(We)
                bias_all[i, :, mat * NDC:(mat + 1) * NDC] = be.reshape(NDC, 128).T
        wo_w[i] = prep_lhsT(aw[i, 3])
        wo_be = ab[i, 3] + bv @ aw[i, 3]
        bias_all[i, :, 16:24] = wo_be.reshape(NDC, 128).T
        g2, b2 = ln_g[i, 1][:, None], ln_b[i, 1]
        W1e = g2 * fw1[i]
        b1e = fb1[i] + b2 @ fw1[i]
        w1_w[i] = prep_lhsT(W1e)
        bias_all[i, :, 24:56] = b1e.reshape(NFC, 128).T
        w2_w[i] = prep_lhsT(fw2[i])
        bias_all[i, :, 56:64] = fb2[i].reshape(NDC, 128).T

    g['qkv_w'] = qkv_w; g['wv_nat'] = wv_nat; g['wo_w'] = wo_w
    g['w1_w'] = w1_w; g['w2_w'] = w2_w; g['bias_all'] = bias_all

    inv = 1.0 / math.sqrt(1.0 + 1e-5)
    fin_g = inputs['fin_g'].astype(f64); fin_b = inputs['fin_b'].astype(f64)
    A1 = fin_g * inv * inputs['cf_bn_g'].astype(f64)
    C1 = fin_b * inv * inputs['cf_bn_g'].astype(f64) + inputs['cf_bn_b'].astype(f64)
    cfw = inputs['cf_w'].astype(f64)
    cf_we = A1[:, None] * cfw
    cf_be = inputs['cf_b'].astype(f64) + C1 @ cfw
    g['cf_w'] = prep_lhsT(cf_we)
    g['cf_bT'] = cf_be.reshape(NDC, 128).T.astype(np.float32)
    A2 = inv * inputs['fc_bn_g'].astype(f64)
    C2 = inputs['fc_bn_b'].astype(f64)
    fcw = inputs['fc_w'].astype(f64)
    fc_we = A2[:, None] * fcw
    fc_be = inputs['fc_b'].astype(f64) + C2 @ fcw
    g['fc_w'] = np.ascontiguousarray(
        fc_we.reshape(NDC, 128, NCLS).transpose(1, 0, 2).reshape(128, NDC * NCLS)
    ).astype(bf16)
    g['fc_b'] = fc_be.reshape(NCLS, 1).astype(np.float32)
    g['ones'] = np.ones((128, 512), np.float32)
    selm = np.zeros((1, 256), np.float32)
    selm[0, 0:64] = 1.0
    selm[0, 192:256] = 1.0
    g['sel'] = selm
    return g


def _run_timed(nc, in_maps, n_iters=10):
    """Mirror bass2jax.run_bass_via_pjrt (no donation), time steady-state execs."""
    import time
    import jax
    import numpy as _np
    from jax.experimental.shard_map import shard_map
    from jax.sharding import Mesh, PartitionSpec, NamedSharding
    from concourse import bass2jax as b2j
    from concourse import mybir as _mb

    b2j.install_neuronx_cc_hook()
    n_cores = len(in_maps)
    partition_name = nc.partition_id_tensor.name if nc.partition_id_tensor else None
    in_names, out_names, out_avals, zero_outs = [], [], [], []
    for alloc in nc.m.functions[0].allocations:
        if not isinstance(alloc, _mb.MemoryLocationSet):
            continue
        name = alloc.memorylocations[0].name
        if alloc.kind == "ExternalInput":
            if name != partition_name:
                in_names.append(name)
        elif alloc.kind == "ExternalOutput":
            shape = tuple(alloc.tensor_shape)
            dtype = _mb.dt.np(alloc.dtype)
            out_names.append(name)
            out_avals.append(jax.core.ShapedArray(shape, dtype))
            zero_outs.append(_np.zeros(shape, dtype))
    n_params = len(in_names)
    all_in_names = list(in_names) + list(out_names)
    if partition_name is not None:
        all_in_names.append(partition_name)

    def _body(*args):
        operands = list(args)
        if partition_name is not None:
            operands.append(b2j.partition_id_tensor())
        outs = b2j._bass_exec_p.bind(
            *operands,
            out_avals=tuple(out_avals),
            in_names=tuple(all_in_names),
            out_names=tuple(out_names),
            lowering_input_output_aliases=(),
            sim_require_finite=True,
            sim_require_nnan=True,
            nc=nc,
        )
        return tuple(outs)

    devices = jax.devices()[:n_cores]
    mesh = Mesh(_np.asarray(devices), ("core",))
    spec = PartitionSpec("core")
    sharded = jax.jit(shard_map(
        _body, mesh=mesh, in_specs=(spec,) * (n_params + len(out_names)),
        out_specs=(spec,) * len(out_names), check_rep=False))
    sh = NamedSharding(mesh, spec)
    concat_in = [
        jax.device_put(_np.concatenate([_np.asarray(m[name]) for m in in_maps], axis=0), sh)
        for name in in_names
    ]
    concat_zeros = [
        jax.device_put(_np.zeros((n_cores * z.shape[0], *z.shape[1:]), z.dtype), sh)
        for z in zero_outs
    ]
    # Warm up (NEFF load, IRAM population, DMA ring setup land here).
    for _ in range(6):
        outs = sharded(*concat_in, *concat_zeros)
    jax.block_until_ready(outs)
    # The dispatch path has a large constant pipeline latency (~70ms RPC
    # round-trip) that is unrelated to device execution: a burst of n calls
    # completes in  T(n) = latency + n * per_call.  Estimate per_call as the
    # marginal cost between a short and a long burst so the constant
    # transport latency cancels out.
    t0 = time.time()
    for _ in range(3):
        outs = sharded(*concat_in, *concat_zeros)
    jax.block_until_ready(outs)
    t1 = time.time()
    for _ in range(n_iters + 3):
        outs = sharded(*concat_in, *concat_zeros)
    jax.block_until_ready(outs)
    t2 = time.time()
    per_call_ns = ((t2 - t1) - (t1 - t0)) / n_iters * 1e9
    results = [
        {name: _np.asarray(outs[i]).reshape(n_cores, *out_avals[i].shape)[c]
         for i, name in enumerate(out_names)}
        for c in range(n_cores)
    ]
    return results, per_call_ns


def kernel(**inputs):
    global LAST_EXEC_NS
    import ml_dtypes
    bf16 = ml_dtypes.bfloat16
    n_layers = int(inputs.pop('_n_layers', L))
    if n_layers not in _CACHE:
        _CACHE[n_layers] = _build(n_layers)
    nc = _CACHE[n_layers]
    g = _prep_weights(inputs, n_layers)

    x = inputs['x']
    xr = x.reshape(B, S, NI)
    in_maps = []
    for ci in range(NCORES):
        xc = xr[ci * NB:(ci + 1) * NB].astype(np.float32)  # [16, 50, 1200]
        xa = np.zeros((NB, S, NIP), np.float32)
        xa[:, :, :NI] = xc
        xa[np.arange(NB)[:, None], np.arange(S)[None, :], NI + np.arange(S)[None, :]] = 1.0
        # xT [NIP, T]: feature-major, tokens (b, s)
        xT = np.ascontiguousarray(xa.reshape(T, NIP).T).astype(bf16)
        m = dict(g)
        m['xT'] = xT
        in_maps.append(m)

    if TRACE:
        results, per_call_ns = _run_timed(nc, in_maps)
        LAST_EXEC_NS = int(per_call_ns)
    else:
        res = run_bass_kernel_spmd(nc, in_maps, core_ids=list(range(NCORES)))
        LAST_EXEC_NS = res.exec_time_ns
        results = res.results
    outs = [r['out'].T for r in results]   # each [NB, NCLS]
    return np.concatenate(outs, axis=0).astype(np.float32)
